# revision 1
# baseline (speedup 1.0000x reference)
"""MemoryRetriever kernel for 8x Trainium2 NeuronCores.

Data-parallel over the B*S=8192 query rows (1024 rows/core); the selected
memory bank and all weights are replicated. All heavy matmuls run in fp32r
(fp32 rounded to 11-bit mantissa, 1 PE cycle/row at free-dim 512).

Device activations live feature-major: [feature partition chunks of 128, rows].

Host-side linear-algebra fusions (exact up to fp32 rounding):
  Q = x @ (wq_in @ Wq).T + (wq_in @ bq + bqi)
  mem-layernorm gamma/beta are folded into wk/wv/bk/bv
  attn_out = ctx @ out_w.T + out_b is folded into the gate/integration
  weights:  cat @ W.T = x @ Wx.T + ctx @ (Wa @ out_w).T + (Wa @ out_b + b)
so the device never materializes attn_out; cat == [x; ctx].

Weights are passed in chunk-contiguous layout [OUTC, 128, INC, ow] so every
weight DMA reads 4-8KB contiguous per partition.
"""

import sys
from contextlib import ExitStack

if "/opt/trn_rl_repo" not in sys.path:
    sys.path.insert(0, "/opt/trn_rl_repo")

import numpy as np

import concourse.bass as bass
import concourse.mybir as mybir
import concourse.tile as tile
from concourse import bacc
from concourse.bass_utils import run_bass_kernel_spmd
from concourse.masks import make_identity

F32 = mybir.dt.float32
F32R = mybir.dt.float32r
AF = mybir.ActivationFunctionType
OP = mybir.AluOpType

H = 1024
NH = 4
HD = H // NH          # 256
K = 2048              # top_k
B, S = 4, 2048
N_CORES = 8
R = (B * S) // N_CORES  # 1024 rows per core
EPS = 1e-5
H2 = 2 * H            # 2048

HC = H // 128         # 8 feature chunks
H2C = H2 // 128       # 16
KC = K // 128         # 16 key chunks
RT = R // 512         # 2 row tiles of 512
KT4 = K // 512        # 4 key tiles of 512


def build_program():
    nc = bacc.Bacc("TRN2", target_bir_lowering=False)

    d_xt = nc.declare_dram_parameter("x_t", [H, R], F32R, isOutput=False)
    d_mem = nc.declare_dram_parameter("mem_t", [H, K], F32R, isOutput=False)
    d_wc = nc.declare_dram_parameter("wc_t", [HC, 128, HC, 128], F32R,
                                     isOutput=False)
    d_wk = nc.declare_dram_parameter("wk_t", [HC, 128, HC, 128], F32R,
                                     isOutput=False)
    d_wv = nc.declare_dram_parameter("wv_t", [2, 128, HC, 512], F32R,
                                     isOutput=False)
    d_gw = nc.declare_dram_parameter("gw_t", [HC, 128, H2C, 128], F32R,
                                     isOutput=False)
    d_w1 = nc.declare_dram_parameter("w1_t", [H2C, 128, H2C, 128], F32R,
                                     isOutput=False)
    d_w2 = nc.declare_dram_parameter("w2_t", [HC, 128, H2C, 128], F32R,
                                     isOutput=False)
    d_bc = nc.declare_dram_parameter("bc", [H], F32, isOutput=False)
    d_bk = nc.declare_dram_parameter("bk", [H], F32, isOutput=False)
    d_bv = nc.declare_dram_parameter("bv", [H], F32, isOutput=False)
    d_gb = nc.declare_dram_parameter("gate_b", [H], F32, isOutput=False)
    d_b1 = nc.declare_dram_parameter("int_b1", [H2], F32, isOutput=False)
    d_b2 = nc.declare_dram_parameter("int_b2", [H], F32, isOutput=False)
    d_ilg = nc.declare_dram_parameter("iln_g", [H2], F32, isOutput=False)
    d_ilb = nc.declare_dram_parameter("iln_b", [H2], F32, isOutput=False)
    d_l2g = nc.declare_dram_parameter("ln2_g", [H], F32, isOutput=False)
    d_l2b = nc.declare_dram_parameter("ln2_b", [H], F32, isOutput=False)
    d_out = nc.declare_dram_parameter("out", [R, H], F32, isOutput=True)

    # DRAM roundtrip for K/V (SBUF can't hold them alongside everything else)
    d_ktd = nc.dram_tensor("ktD", [H, K], F32R)
    d_vd = nc.dram_tensor("vD", [K, H], F32R)

    with tile.TileContext(nc) as tc, ExitStack() as top:
        singles = top.enter_context(tc.tile_pool(name="singles", bufs=1))

        ident = singles.tile([128, 128], F32)
        make_identity(nc, ident)
        scratch1 = singles.tile([128, 128], F32)
        nc.vector.memset(scratch1, 1.0)
        ones_sm = singles.tile([128, 128], F32R)
        nc.scalar.activation(out=ones_sm, in_=scratch1, func=AF.Copy)
        ones_1k = singles.tile([128, 128], F32R)
        nc.scalar.activation(out=ones_1k, in_=scratch1, func=AF.Copy,
                             scale=1.0 / 1024.0)
        ones_2k = singles.tile([128, 128], F32R)
        nc.scalar.activation(out=ones_2k, in_=scratch1, func=AF.Copy,
                             scale=1.0 / 2048.0)
        eps_t = singles.tile([128, 1], F32)
        nc.vector.memset(eps_t, EPS)

        def load_pp(vec, n, nm):  # [n*128] dram vector -> [128, n] per-partition
            t = singles.tile([128, n], F32, tag=f"pp_{nm}", name=f"pp_{nm}")
            nc.sync.dma_start(out=t, in_=vec[:].rearrange("(c p) -> p c", p=128))
            return t

        # =========== Phase A: mem layernorm + K/V projections ===========
        with ExitStack() as sa:
            pa = sa.enter_context(tc.tile_pool(name="pa", bufs=1))
            pa_sq = sa.enter_context(tc.tile_pool(name="pa_sq", bufs=2))
            mem_sb = pa.tile([128, HC, K], F32R)  # 8 MB
            for hc in range(HC):
                nc.sync.dma_start(out=mem_sb[:, hc, :],
                                  in_=d_mem[hc * 128:(hc + 1) * 128, :])
            mu_sb = pa.tile([128, K], F32)
            rstd_sb = pa.tile([128, K], F32)
            bc_sb = load_pp(d_bc, HC, "bc")
            bk_sb = load_pp(d_bk, HC, "bk")
            bv_sb = load_pp(d_bv, HC, "bv")
            gb_sb = load_pp(d_gb, HC, "gb")
            b1_sb = load_pp(d_b1, H2C, "b1")
            b2_sb = load_pp(d_b2, HC, "b2")
            ilg_sb = load_pp(d_ilg, H2C, "ilg")
            ilb_sb = load_pp(d_ilb, H2C, "ilb")
            # stats: mean / mean-square over the 1024 features (partition dim)
            with tc.tile_pool(name="pa_st", bufs=1, space="PSUM") as pa_st:
                mu_ps = [pa_st.tile([128, 512], F32, tag=f"mu{i}", name=f"mu{i}")
                         for i in range(KT4)]
                ms_ps = [pa_st.tile([128, 512], F32, tag=f"ms{i}", name=f"ms{i}")
                         for i in range(KT4)]
                for hc in range(HC):
                    sq = pa_sq.tile([128, K], F32R, tag="sqt1", name="sq")
                    nc.vector.tensor_mul(sq, mem_sb[:, hc, :].bitcast(F32),
                                         mem_sb[:, hc, :].bitcast(F32))
                    for i in range(KT4):
                        sl = bass.ts(i, 512)
                        nc.tensor.matmul(mu_ps[i], ones_1k, mem_sb[:, hc, sl],
                                         start=(hc == 0), stop=(hc == HC - 1))
                        nc.tensor.matmul(ms_ps[i], ones_1k, sq[:, sl],
                                         start=(hc == 0), stop=(hc == HC - 1))
                for i in range(KT4):
                    sl = bass.ts(i, 512)
                    nc.scalar.activation(out=mu_sb[:, sl], in_=mu_ps[i],
                                         func=AF.Copy)
                    var = pa_sq.tile([128, 512], F32, tag="var", name="var")
                    nc.vector.tensor_mul(var, mu_sb[:, sl], mu_sb[:, sl])
                    nc.vector.tensor_sub(var, ms_ps[i], var)
                    # rstd = exp(-0.5*ln(var+eps)); Ln/Exp share one table set
                    nc.scalar.activation(out=var, in_=var, func=AF.Ln,
                                         bias=eps_t, scale=1.0)
                    nc.scalar.activation(out=rstd_sb[:, sl], in_=var,
                                         func=AF.Exp, scale=-0.5)
            # apply LN in place (f32r); ln1 gamma/beta folded into wk/wv on host
            # per 512-wide tile so the K projection can start on tile 0 early
            for i in range(KT4):
                sl = bass.ts(i, 512)
                for hc in range(HC):
                    t1 = pa_sq.tile([128, 512], F32, tag="sqt1", name="t1")
                    nc.vector.tensor_sub(t1, mem_sb[:, hc, sl].bitcast(F32),
                                         mu_sb[:, sl])
                    nc.vector.tensor_mul(mem_sb[:, hc, sl], t1, rstd_sb[:, sl])
            # K_t = wk.T-matmul(mem_n) + bk  -> dram ktD [H, K]
            with ExitStack() as skv:
                pa_w = skv.enter_context(tc.tile_pool(name="pa_w", bufs=2))
                pa_o = skv.enter_context(tc.tile_pool(name="pa_o", bufs=4))
                pa_ps = skv.enter_context(
                    tc.tile_pool(name="pa_ps", bufs=2, space="PSUM"))
                for oc in range(HC):
                    wks = pa_w.tile([128, HC, 128], F32R, tag="wk", name="wks", bufs=3)
                    nc.sync.dma_start(out=wks, in_=d_wk[oc])
                    for i in range(KT4):
                        sl = bass.ts(i, 512)
                        ps = pa_ps.tile([128, 512], F32, tag="kps", name="kps")
                        for hc in range(HC):
                            nc.tensor.matmul(ps, wks[:, hc, :], mem_sb[:, hc, sl],
                                             start=(hc == 0), stop=(hc == HC - 1))
                        ko = pa_o.tile([128, 512], F32R, tag="ko", name="ko")
                        nc.scalar.activation(out=ko, in_=ps, func=AF.Identity,
                                             bias=bk_sb[:, oc:oc + 1])
                        nc.sync.dma_start(out=d_ktd[oc * 128:(oc + 1) * 128, sl],
                                          in_=ko)
                # V = mem_n @ wv.T (bias bv folded after softmax) -> dram vD [K,H]
                for ot in range(2):
                    osl = bass.ts(ot, 512)
                    wvs = pa_w.tile([128, HC, 512], F32R, tag="wv", name="wvs")
                    nc.sync.dma_start(out=wvs, in_=d_wv[ot])
                    for kc in range(KC):
                        ps = pa_ps.tile([128, 512], F32, tag="vps", name="vps")
                        for hc in range(HC):
                            nc.tensor.matmul(
                                ps, mem_sb[:, hc, kc * 128:(kc + 1) * 128],
                                wvs[:, hc, :],
                                start=(hc == 0), stop=(hc == HC - 1))
                        vo = pa_o.tile([128, 512], F32R, tag="vo", name="vo")
                        nc.scalar.activation(out=vo, in_=ps, func=AF.Copy)
                        nc.sync.dma_start(
                            out=d_vd[kc * 128:(kc + 1) * 128, osl], in_=vo)

        xt_sb = singles.tile([128, HC, R], F32R)   # resident until the end
        for hc in range(HC):
            nc.sync.dma_start(out=xt_sb[:, hc, :],
                              in_=d_xt[hc * 128:(hc + 1) * 128, :])

        # =========== Phases B+C: query projection + attention ===========
        with ExitStack() as sbc:
            pct = sbc.enter_context(tc.tile_pool(name="pct", bufs=1))
            ctxt_sb = pct.tile([128, HC, R], F32R)
            with ExitStack() as spq:
                pq = spq.enter_context(tc.tile_pool(name="pq", bufs=1))
                qt_sb = pq.tile([128, HC, R], F32R)
                with ExitStack() as sb_:
                    pb_w = sb_.enter_context(tc.tile_pool(name="pb_w", bufs=3))
                    pb_ps = sb_.enter_context(
                        tc.tile_pool(name="pb_ps", bufs=4, space="PSUM"))
                    for oc in range(HC):
                        wcs = pb_w.tile([128, HC, 128], F32R, tag="wc",
                                        name="wcs")
                        nc.sync.dma_start(out=wcs, in_=d_wc[oc])
                        for rt in range(RT):
                            sl = bass.ts(rt, 512)
                            ps = pb_ps.tile([128, 512], F32, tag="qps",
                                            name="qps")
                            for hc in range(HC):
                                nc.tensor.matmul(ps, wcs[:, hc, :],
                                                 xt_sb[:, hc, sl],
                                                 start=(hc == 0),
                                                 stop=(hc == HC - 1))
                            nc.scalar.activation(out=qt_sb[:, oc, sl], in_=ps,
                                                 func=AF.Identity,
                                                 bias=bc_sb[:, oc:oc + 1])

                with ExitStack() as sc_:
                    pc_kv = sc_.enter_context(tc.tile_pool(name="pc_kv", bufs=2))
                    pc_e = sc_.enter_context(tc.tile_pool(name="pc_e", bufs=6))
                    pc_o = sc_.enter_context(tc.tile_pool(name="pc_o", bufs=4))
                    pc_sc = sc_.enter_context(
                        tc.tile_pool(name="pc_sc", bufs=2, space="PSUM"))
                    pc_acc = sc_.enter_context(
                        tc.tile_pool(name="pc_acc", bufs=2, space="PSUM"))
                    for h in range(NH):
                        kh = pc_kv.tile([128, 2, K], F32R, tag="kh", name="kh")
                        for j in range(2):
                            row0 = h * HD + j * 128
                            nc.sync.dma_start(out=kh[:, j, :],
                                              in_=d_ktd[row0:row0 + 128, :])
                        vh = pc_kv.tile([128, KC, HD], F32R, tag="vh", name="vh")
                        for kc in range(KC):
                            nc.sync.dma_start(
                                out=vh[:, kc, :],
                                in_=d_vd[kc * 128:(kc + 1) * 128,
                                         h * HD:(h + 1) * HD])
                        for qt in range(RT):
                            qsl = bass.ts(qt, 512)
                            sums = pc_acc.tile([128, 512], F32, tag="sums",
                                               name="sums")
                            ctx0 = pc_acc.tile([128, 512], F32, tag="ctx0",
                                               name="ctx0")
                            ctx1 = pc_acc.tile([128, 512], F32, tag="ctx1",
                                               name="ctx1")
                            for kt in range(KC):
                                sc = pc_sc.tile([128, 512], F32, tag="sc",
                                                name="sc")
                                for j in range(2):
                                    nc.tensor.matmul(
                                        sc, kh[:, j, kt * 128:(kt + 1) * 128],
                                        qt_sb[:, h * 2 + j, qsl],
                                        start=(j == 0), stop=(j == 1))
                                e = pc_e.tile([128, 512], F32R, tag="e",
                                              name="e")
                                nc.scalar.activation(out=e, in_=sc, func=AF.Exp,
                                                     scale=1.0 / 16.0)
                                nc.tensor.matmul(sums, ones_sm, e,
                                                 start=(kt == 0),
                                                 stop=(kt == KC - 1))
                                nc.tensor.matmul(ctx0, vh[:, kt, 0:128], e,
                                                 start=(kt == 0),
                                                 stop=(kt == KC - 1))
                                nc.tensor.matmul(ctx1, vh[:, kt, 128:256], e,
                                                 start=(kt == 0),
                                                 stop=(kt == KC - 1))
                            rec = pc_o.tile([128, 512], F32, tag="rec",
                                            name="rec")
                            nc.vector.reciprocal(out=rec, in_=sums)
                            for j, ctx in enumerate((ctx0, ctx1)):
                                tmp = pc_o.tile([128, 512], F32, tag="ctmp",
                                                name="ctmp")
                                nc.vector.tensor_mul(tmp, ctx, rec)
                                nc.scalar.activation(
                                    out=ctxt_sb[:, h * 2 + j, qsl], in_=tmp,
                                    func=AF.Identity,
                                    bias=bv_sb[:, h * 2 + j:h * 2 + j + 1])

            # =========== Phase D: gated integration MLP ===========
            # cat == [x ; ctx]  (out_w folded into gate/int weights on host)
            def cat_chunk(hc):
                return xt_sb[:, hc, :] if hc < HC else ctxt_sb[:, hc - HC, :]

            pd_w2 = sbc.enter_context(tc.tile_pool(name="pd_w2", bufs=2))
            l2g_bc = singles.tile([128, H], F32)
            nc.sync.dma_start(
                out=l2g_bc,
                in_=d_l2g[:].unsqueeze(0).partition_broadcast(128).squeeze(1))
            l2b_bc = singles.tile([128, H], F32)
            nc.sync.dma_start(
                out=l2b_bc,
                in_=d_l2b[:].unsqueeze(0).partition_broadcast(128).squeeze(1))
            with ExitStack() as sd:
                pd = sd.enter_context(tc.tile_pool(name="pd", bufs=1))
                h1_sb = pd.tile([128, H2C, R], F32R)   # 8 MB
                with ExitStack() as sd12:
                    pd_st = sd12.enter_context(tc.tile_pool(name="pd_st",
                                                            bufs=1))
                    mu2_sb = pd_st.tile([128, R], F32)
                    rstd2_sb = pd_st.tile([128, R], F32)
                    pd_w1 = sd12.enter_context(tc.tile_pool(name="pd_w1",
                                                            bufs=3))
                    pd_sq = sd12.enter_context(tc.tile_pool(name="pd_sq",
                                                            bufs=2))
                    pd_ps = sd12.enter_context(
                        tc.tile_pool(name="pd_ps", bufs=1, space="PSUM"))
                    h1ps = [pd_ps.tile([128, 512], F32, tag=f"h1ps{i}",
                                       name=f"h1ps{i}") for i in range(4)]
                    for oc2 in range(H2C):
                        w1s = pd_w1.tile([128, H2C, 128], F32R, tag="w1",
                                         name="w1s")
                        nc.sync.dma_start(out=w1s, in_=d_w1[oc2])
                        for rt in range(RT):
                            sl = bass.ts(rt, 512)
                            ps = h1ps[(oc2 * RT + rt) % 4]
                            for hc in range(H2C):
                                nc.tensor.matmul(ps, w1s[:, hc, :],
                                                 cat_chunk(hc)[:, sl],
                                                 start=(hc == 0),
                                                 stop=(hc == H2C - 1))
                            nc.scalar.activation(out=h1_sb[:, oc2, sl], in_=ps,
                                                 func=AF.Identity,
                                                 bias=b1_sb[:, oc2:oc2 + 1])
                    # D2: layernorm over 2048 features + exact gelu (in place)
                    mu2_ps = [pd_ps.tile([128, 512], F32, tag=f"m2_{i}",
                                         name=f"m2_{i}") for i in range(RT)]
                    ms2_ps = [pd_ps.tile([128, 512], F32, tag=f"s2_{i}",
                                         name=f"s2_{i}") for i in range(RT)]
                    for oc2 in range(H2C):
                        sq = pd_sq.tile([128, R], F32R, tag="sqt1", name="sq2")
                        nc.vector.tensor_mul(sq, h1_sb[:, oc2, :].bitcast(F32),
                                             h1_sb[:, oc2, :].bitcast(F32))
                        for i in range(RT):
                            sl = bass.ts(i, 512)
                            nc.tensor.matmul(mu2_ps[i], ones_2k,
                                             h1_sb[:, oc2, sl],
                                             start=(oc2 == 0),
                                             stop=(oc2 == H2C - 1))
                            nc.tensor.matmul(ms2_ps[i], ones_2k, sq[:, sl],
                                             start=(oc2 == 0),
                                             stop=(oc2 == H2C - 1))
                    for i in range(RT):
                        sl = bass.ts(i, 512)
                        nc.scalar.activation(out=mu2_sb[:, sl], in_=mu2_ps[i],
                                             func=AF.Copy)
                        var = pd_sq.tile([128, 512], F32, tag="var2",
                                         name="var2")
                        nc.vector.tensor_mul(var, mu2_sb[:, sl], mu2_sb[:, sl])
                        nc.vector.tensor_sub(var, ms2_ps[i], var)
                        nc.scalar.activation(out=var, in_=var, func=AF.Ln,
                                             bias=eps_t, scale=1.0)
                        nc.scalar.activation(out=rstd2_sb[:, sl], in_=var,
                                             func=AF.Exp, scale=-0.5)
                    for oc2 in range(H2C):
                        t1 = pd_sq.tile([128, R], F32, tag="sqt1", name="t1d")
                        nc.vector.tensor_sub(t1, h1_sb[:, oc2, :].bitcast(F32),
                                             mu2_sb)
                        nc.vector.scalar_tensor_tensor(
                            out=t1, in0=t1, scalar=ilg_sb[:, oc2:oc2 + 1],
                            in1=rstd2_sb, op0=OP.mult, op1=OP.mult)
                        nc.scalar.activation(out=h1_sb[:, oc2, :], in_=t1,
                                             func=AF.Gelu,
                                             bias=ilb_sb[:, oc2:oc2 + 1])
                # D3: integ = gelu(h1) @ w2.T + b2; gate = sigmoid(cat@gw.T+gb)
                #     y = x + gate * integ         (feature-major, fp32)
                with ExitStack() as sd34:
                    pd_y = sd34.enter_context(tc.tile_pool(name="pd_y", bufs=1))
                    yt_sb = pd_y.tile([128, HC, R], F32)
                    pd_o = sd34.enter_context(tc.tile_pool(name="pd_o", bufs=2))
                    pd_yr = sd34.enter_context(tc.tile_pool(name="pd_yr",
                                                            bufs=2))
                    pd_ps3 = sd34.enter_context(
                        tc.tile_pool(name="pd_ps3", bufs=2, space="PSUM"))
                    pd_ps4 = sd34.enter_context(
                        tc.tile_pool(name="pd_ps4", bufs=2, space="PSUM"))

                    def d4_chunk(rc):
                        tp = pd_ps4.tile([128, 1024], F32, tag="tp", name="tp")
                        for oc in range(HC):
                            nc.tensor.transpose(
                                tp[:, oc * 128:(oc + 1) * 128],
                                yt_sb[:, oc, rc * 128:(rc + 1) * 128], ident)
                        yr = pd_yr.tile([128, H], F32, tag="yr", name="yr")
                        nc.scalar.activation(out=yr[:, 0:512], in_=tp[:, 0:512],
                                             func=AF.Copy)
                        nc.scalar.activation(out=yr[:, 512:1024],
                                             in_=tp[:, 512:1024], func=AF.Copy)
                        stats = pd_o.tile([128, 2, 6], F32, tag="bst",
                                          name="bst")
                        for i in range(2):
                            nc.vector.bn_stats(out=stats[:, i, :],
                                               in_=yr[:, i * 512:(i + 1) * 512])
                        mv = pd_o.tile([128, 2], F32, tag="mv", name="mv")
                        nc.vector.bn_aggr(out=mv, in_=stats)
                        sd_ = pd_o.tile([128, 1], F32, tag="sd", name="sd")
                        nc.scalar.activation(out=sd_, in_=mv[:, 1:2],
                                             func=AF.Sqrt, bias=eps_t, scale=1.0)
                        rstd = pd_o.tile([128, 1], F32, tag="rsd", name="rstd")
                        nc.vector.reciprocal(out=rstd, in_=sd_)
                        nmr = pd_o.tile([128, 1], F32, tag="nmr", name="nmr")
                        nc.vector.scalar_tensor_tensor(
                            out=nmr, in0=mv[:, 0:1], scalar=-1.0, in1=rstd,
                            op0=OP.mult, op1=OP.mult)
                        nc.scalar.activation(out=yr, in_=yr, func=AF.Identity,
                                             bias=nmr, scale=rstd)
                        nc.vector.tensor_mul(yr, yr, l2g_bc)
                        nc.vector.tensor_add(yr, yr, l2b_bc)
                        nc.sync.dma_start(out=d_out[rc * 128:(rc + 1) * 128, :],
                                          in_=yr)

                    for rt in range(RT):
                        sl = bass.ts(rt, 512)
                        for oc in range(HC):
                            w2s = pd_w2.tile([128, H2C, 128], F32R, tag="w23",
                                             name="w2s")
                            gws = pd_w2.tile([128, H2C, 128], F32R, tag="w23",
                                             name="gws")
                            nc.sync.dma_start(out=gws, in_=d_gw[oc])
                            nc.sync.dma_start(out=w2s, in_=d_w2[oc])
                            gps = pd_ps3.tile([128, 512], F32, tag="gps",
                                              name="gps")
                            for hc in range(H2C):
                                nc.tensor.matmul(gps, gws[:, hc, :],
                                                 cat_chunk(hc)[:, sl],
                                                 start=(hc == 0),
                                                 stop=(hc == H2C - 1))
                            igps = pd_ps3.tile([128, 512], F32, tag="igps",
                                               name="igps")
                            for hc in range(H2C):
                                nc.tensor.matmul(igps, w2s[:, hc, :],
                                                 h1_sb[:, hc, sl],
                                                 start=(hc == 0),
                                                 stop=(hc == H2C - 1))
                            sig = pd_o.tile([128, 512], F32, tag="sig",
                                            name="sig", bufs=4)
                            nc.scalar.activation(out=sig, in_=gps,
                                                 func=AF.Sigmoid,
                                                 bias=gb_sb[:, oc:oc + 1])
                            tmp = pd_o.tile([128, 512], F32, tag="ytmp",
                                            name="ytmp")
                            nc.vector.scalar_tensor_tensor(
                                out=tmp, in0=igps, scalar=b2_sb[:, oc:oc + 1],
                                in1=sig, op0=OP.add, op1=OP.mult)
                            nc.vector.tensor_add(yt_sb[:, oc, sl], tmp,
                                                 xt_sb[:, oc, sl].bitcast(F32))
                        for rc in range(rt * 4, rt * 4 + 4):
                            d4_chunk(rc)

    nc.compile()
    return nc


_NC_CACHE = []


def _get_nc():
    if not _NC_CACHE:
        _NC_CACHE.append(build_program())
    return _NC_CACHE[0]


def kernel(query_hidden, mem_keys, importance, recency, access_count,
           Wq, bq, in_w, in_b, out_w, out_b, gate_w, gate_b,
           int_w1, int_b1, int_ln_g, int_ln_b, int_w2, int_b2,
           ln1_g, ln1_b, ln2_g, ln2_b, sel_params, top_k):
    np32 = lambda a: np.asarray(a, dtype=np.float32)
    query_hidden = np32(query_hidden)
    mem_keys = np32(mem_keys)
    top_k = int(top_k)
    assert top_k == K, f"kernel compiled for top_k={K}, got {top_k}"

    # HTPS selection (host): softmax-weighted score, top-k set, gather.
    # Attention output is invariant to the order of the selected rows, so an
    # argpartition set (== jax.lax.top_k set) is sufficient.
    sp = np32(sel_params)
    w = np.exp(sp - sp.max())
    w = w / w.sum()
    acc = np32(access_count)
    sel = w[0] * np32(importance) + w[1] * np32(recency) + w[2] * (acc / acc.max())
    idx = np.argpartition(-sel, top_k - 1)[:top_k]
    mem_t = np.ascontiguousarray(mem_keys[idx].T)      # [H, K]

    in_w = np32(in_w)
    in_b = np32(in_b)
    wq, wk, wv = in_w[:H], in_w[H:2 * H], in_w[2 * H:]
    bqi, bki, bvi = in_b[:H], in_b[H:2 * H], in_b[2 * H:]
    wc = wq @ np32(Wq)                                  # fused query projection
    bc = wq @ np32(bq) + bqi

    # fold mem-layernorm gamma/beta into the K/V projections
    g1 = np32(ln1_g)
    b1v = np32(ln1_b)
    bki = bki + wk @ b1v
    bvi = bvi + wv @ b1v
    wk = wk * g1[None, :]
    wv = wv * g1[None, :]

    # fold attn_out = ctx @ out_w.T + out_b into the gate / integration weights
    out_w = np32(out_w)
    out_b = np32(out_b)
    gate_w = np32(gate_w)
    int_w1 = np32(int_w1)
    gwx, gwa = gate_w[:, :H], gate_w[:, H:]
    w1x, w1a = int_w1[:, :H], int_w1[:, H:]
    gate_b_f = np32(gate_b) + gwa @ out_b
    int_b1_f = np32(int_b1) + w1a @ out_b

    T = lambda a: np.ascontiguousarray(np32(a).T)

    def chunked(w_t, ow=128):
        # [IN, OUT] -> [OUT//ow, 128, IN//128, ow]: contiguous per-partition slabs
        inn, out = w_t.shape
        r = w_t.reshape(inn // 128, 128, out // ow, ow).transpose(2, 1, 0, 3)
        return np.ascontiguousarray(r)

    gw_t = np.concatenate([gwx.T, (gwa @ out_w).T], axis=0)
    w1_t = np.concatenate([w1x.T, (w1a @ out_w).T], axis=0)

    common = {
        "mem_t": mem_t,
        "wc_t": chunked(T(wc)), "wk_t": chunked(T(wk)),
        "wv_t": chunked(T(wv), ow=512),
        "gw_t": chunked(gw_t), "w1_t": chunked(w1_t),
        "w2_t": chunked(T(int_w2)),
        "bc": bc, "bk": bki, "bv": bvi,
        "gate_b": gate_b_f, "int_b1": int_b1_f, "int_b2": np32(int_b2),
        "iln_g": np32(int_ln_g), "iln_b": np32(int_ln_b),
        "ln2_g": np32(ln2_g), "ln2_b": np32(ln2_b),
    }
    X = query_hidden.reshape(B * S, H)
    in_maps = []
    for c in range(N_CORES):
        m = dict(common)
        m["x_t"] = np.ascontiguousarray(X[c * R:(c + 1) * R].T)
        in_maps.append(m)

    nc = _get_nc()
    res = run_bass_kernel_spmd(nc, in_maps, core_ids=list(range(N_CORES)))
    out = np.empty((B * S, H), dtype=np.float32)
    for c in range(N_CORES):
        out[c * R:(c + 1) * R] = res.results[c]["out"]
    return out.reshape(B, S, H)



# revision 16
# speedup vs baseline: 1.4211x; 1.4211x over previous
"""MemoryRetriever kernel for 8x Trainium2 NeuronCores.

Data-parallel over the B*S=8192 query rows (1024 rows/core); the selected
memory bank and all weights are replicated.

Precision plan (validated vs reference on CPU, rel err ~1.0e-2 < 2e-2):
  - K/V/Q projections, attention scores/softmax/context, and the sigmoid
    gate run in fp8e4 (e4m3) with DoubleRow matmuls (2 contraction chunks
    of 128 per instruction, 2x fp32r throughput). fp8 weights are
    prescaled x64 on host; Q/K activations are stored x8.
  - The integration path (h1 = cat @ w1, integ = gelu @ w2) runs with
    bf16 stationary weights and f32r/bf16 moving activations (1 PE
    cycle/row), since fp8 there pushes the error over tolerance.

Host-side linear-algebra fusions (exact up to rounding):
  Q = x @ (wq_in @ Wq).T + (wq_in @ bq + bqi)
  memory layernorm (stats + apply + ln1 gamma/beta fold) done on host;
  device receives mem_n directly as fp8.
  attn_out = ctx @ out_w.T + out_b folded into the gate/integration
  weights, so cat == [x; ctx] on device.

Device layout is feature-major: [feature chunk of 128 partitions, rows].
K and V stay SBUF-resident (fp8, 2 MB each) - no DRAM roundtrip.
"""

import sys
from contextlib import ExitStack

if "/opt/trn_rl_repo" not in sys.path:
    sys.path.insert(0, "/opt/trn_rl_repo")

import numpy as np
import ml_dtypes

import concourse.bass as bass
import concourse.mybir as mybir
import concourse.tile as tile
from concourse import bacc
from concourse.bass_utils import run_bass_kernel_spmd
from concourse.masks import make_identity

F32 = mybir.dt.float32
F32R = mybir.dt.float32r
BF16 = mybir.dt.bfloat16
F8 = mybir.dt.float8e4
AF = mybir.ActivationFunctionType
OP = mybir.AluOpType
DR = mybir.MatmulPerfMode.DoubleRow

H = 1024
NH = 4
HD = H // NH          # 256
K = 2048              # top_k
B, S = 4, 2048
N_CORES = 8
R = (B * S) // N_CORES  # 1024 rows per core
EPS = 1e-5
H2 = 2 * H            # 2048

HC = H // 128         # 8 feature chunks
H2C = H2 // 128       # 16
KC = K // 128         # 16 key chunks
RT = R // 512         # 2 row tiles of 512
KT4 = K // 512        # 4 key tiles of 512
WS = 64.0             # fp8 weight prescale
QS = 8.0              # fp8 Q/K activation prescale


def build_program():
    nc = bacc.Bacc("TRN2", target_bir_lowering=False)

    d_xt = nc.declare_dram_parameter("x_t", [H, R], F32R, isOutput=False)
    d_x8 = nc.declare_dram_parameter("x8_t", [H, R], F8, isOutput=False)
    d_mn8 = nc.declare_dram_parameter("mn8_t", [H, K], F8, isOutput=False)
    d_wc = nc.declare_dram_parameter("wc8", [HC, 128, HC, 128], F8,
                                     isOutput=False)
    d_wk = nc.declare_dram_parameter("wk8", [HC, 128, HC, 128], F8,
                                     isOutput=False)
    d_wv = nc.declare_dram_parameter("wv8", [128, HC, H], F8, isOutput=False)
    d_gw = nc.declare_dram_parameter("gw8", [HC, 128, H2C, 128], F8,
                                     isOutput=False)
    d_w1 = nc.declare_dram_parameter("w1b", [H2C, 128, H2C, 128], F32R,
                                     isOutput=False)
    d_w2 = nc.declare_dram_parameter("w2b", [HC, 128, H2C, 128], BF16,
                                     isOutput=False)
    d_bc = nc.declare_dram_parameter("bc8", [H], F32, isOutput=False)
    d_bk = nc.declare_dram_parameter("bk8", [H], F32, isOutput=False)
    d_bv = nc.declare_dram_parameter("bv", [H], F32, isOutput=False)
    d_gb = nc.declare_dram_parameter("gate_b", [H], F32, isOutput=False)
    d_b1 = nc.declare_dram_parameter("int_b1", [H2], F32, isOutput=False)
    d_b2 = nc.declare_dram_parameter("int_b2", [H], F32, isOutput=False)
    d_ilg = nc.declare_dram_parameter("iln_g", [H2], F32, isOutput=False)
    d_ilb = nc.declare_dram_parameter("iln_b", [H2], F32, isOutput=False)
    d_l2g = nc.declare_dram_parameter("ln2_g", [H], F32, isOutput=False)
    d_l2b = nc.declare_dram_parameter("ln2_b", [H], F32, isOutput=False)
    d_out = nc.declare_dram_parameter("out", [R, H], F32, isOutput=True)

    with tile.TileContext(nc) as tc, ExitStack() as top:
        singles = top.enter_context(tc.tile_pool(name="singles", bufs=1))

        ident_f = singles.tile([128, 128], F32)
        make_identity(nc, ident_f)
        ident = singles.tile([128, 128], F32R)
        nc.scalar.activation(out=ident, in_=ident_f, func=AF.Copy)
        scratch1 = singles.tile([128, 128], F32)
        nc.vector.memset(scratch1, 1.0)
        ones8 = singles.tile([128, 2, 128], F8)
        nc.scalar.activation(out=ones8[:, 0], in_=scratch1, func=AF.Copy)
        nc.scalar.activation(out=ones8[:, 1], in_=scratch1, func=AF.Copy)
        ones1k = singles.tile([128, 128], F32R)
        nc.scalar.activation(out=ones1k, in_=scratch1, func=AF.Copy,
                             scale=1.0 / 1024.0)
        ones2kb = singles.tile([128, 128], BF16)
        nc.scalar.activation(out=ones2kb, in_=scratch1, func=AF.Copy,
                             scale=1.0 / 2048.0)
        eps_t = singles.tile([128, 1], F32)
        nc.vector.memset(eps_t, EPS)

        def load_pp(vec, n, nm):  # [n*128] dram vector -> [128, n]
            t = singles.tile([128, n], F32, tag=f"pp_{nm}", name=f"pp_{nm}")
            nc.sync.dma_start(out=t, in_=vec[:].rearrange("(c p) -> p c", p=128))
            return t

        bc8_sb = load_pp(d_bc, HC, "bc")
        bk8_sb = load_pp(d_bk, HC, "bk")
        bv_sb = load_pp(d_bv, HC, "bv")
        gb_sb = load_pp(d_gb, HC, "gb")
        b1_sb = load_pp(d_b1, H2C, "b1")
        b2_sb = load_pp(d_b2, HC, "b2")
        ilg_sb = load_pp(d_ilg, H2C, "ilg")
        ilb_sb = load_pp(d_ilb, H2C, "ilb")
        xt_sb = singles.tile([128, HC, R], F32R)      # 4 MB, residual
        nc.sync.dma_start(
            out=xt_sb, in_=d_xt[:].rearrange("(c p) r -> p c r", p=128))

        # pool lifetimes (stack/LIFO): pDlong outlives pBC outlives pA
        pDlong = top.enter_context(tc.tile_pool(name="pDlong", bufs=1))
        pBC = tc.alloc_tile_pool(name="pBC", bufs=1)  # released after D0

        x8_sb = pBC.tile([128, HC, R], F8)            # 1 MB
        nc.sync.dma_start(
            out=x8_sb, in_=d_x8[:].rearrange("(c p) r -> p c r", p=128))
        ksb = pBC.tile([128, HC, K], F8)              # 2 MB: K^T, 8*K_true
        vsb = pBC.tile([128, KC, H], F8)              # 2 MB: V rows
        q8_sb = pBC.tile([128, HC, R], F8)            # 1 MB: 8*Q
        ctx8_sb = pBC.tile([128, HC, R], F8)          # 1 MB: fp8 ctx
        ctxb_sb = pDlong.tile([128, HC, R], F32R)     # 4 MB: ctx (+bv)

        # =========== Phase A: K/V projections (fp8 DoubleRow) ===========
        with ExitStack() as sa:
            pa_in = sa.enter_context(tc.tile_pool(name="pa_in", bufs=1))
            mn8_sb = pa_in.tile([128, HC, K], F8)     # 2 MB, layernormed mem
            nc.sync.dma_start(
                out=mn8_sb, in_=d_mn8[:].rearrange("(c p) k -> p c k", p=128))
            wv8_sb = pa_in.tile([128, HC, H], F8)     # 1 MB
            nc.sync.dma_start(out=wv8_sb, in_=d_wv[:])
            pa_w = sa.enter_context(tc.tile_pool(name="pa_w", bufs=3))
            pa_ps = sa.enter_context(
                tc.tile_pool(name="pa_ps", bufs=2, space="PSUM"))
            for oc in range(HC):
                wks = pa_w.tile([128, HC, 128], F8, tag="wk", name="wks")
                nc.sync.dma_start(out=wks, in_=d_wk[oc])
                for kt in range(KT4):
                    sl = bass.ts(kt, 512)
                    ps = pa_ps.tile([128, 512], F32, tag="kps", name="kps")
                    for p in range(4):
                        nc.tensor.matmul(ps, wks[:, 2 * p:2 * p + 2, :],
                                         mn8_sb[:, 2 * p:2 * p + 2, sl],
                                         start=(p == 0), stop=(p == 3),
                                         perf_mode=DR)
                    nc.scalar.activation(out=ksb[:, oc, sl], in_=ps,
                                         func=AF.Identity, scale=1.0 / 8.0,
                                         bias=bk8_sb[:, oc:oc + 1])
            for kc in range(KC):
                for half in range(2):
                    hsl = bass.ts(half, 512)
                    ps = pa_ps.tile([128, 512], F32, tag="vps", name="vps")
                    for p in range(4):
                        nc.tensor.matmul(
                            ps,
                            mn8_sb[:, 2 * p:2 * p + 2, kc * 128:(kc + 1) * 128],
                            wv8_sb[:, 2 * p:2 * p + 2, hsl],
                            start=(p == 0), stop=(p == 3), perf_mode=DR)
                    # V stored unscaled: psum = 64*V -> scale 1/64
                    nc.scalar.activation(out=vsb[:, kc, hsl], in_=ps,
                                         func=AF.Copy, scale=1.0 / WS)

        # =========== Phase B: query projection (fp8 DoubleRow) ===========
        with ExitStack() as sb_:
            pb_w = sb_.enter_context(tc.tile_pool(name="pb_w", bufs=3))
            pb_ps = sb_.enter_context(
                tc.tile_pool(name="pb_ps", bufs=2, space="PSUM"))
            for oc in range(HC):
                wcs = pb_w.tile([128, HC, 128], F8, tag="wc", name="wcs")
                nc.sync.dma_start(out=wcs, in_=d_wc[oc])
                for rt in range(RT):
                    sl = bass.ts(rt, 512)
                    ps = pb_ps.tile([128, 512], F32, tag="qps", name="qps")
                    for p in range(4):
                        nc.tensor.matmul(ps, wcs[:, 2 * p:2 * p + 2, :],
                                         x8_sb[:, 2 * p:2 * p + 2, sl],
                                         start=(p == 0), stop=(p == 3),
                                         perf_mode=DR)
                    nc.scalar.activation(out=q8_sb[:, oc, sl], in_=ps,
                                         func=AF.Identity, scale=1.0 / 8.0,
                                         bias=bc8_sb[:, oc:oc + 1])

        # =========== Phase C: attention (fp8 DoubleRow + exp pipeline) =====
        with ExitStack() as sc_:
            pc_e = sc_.enter_context(tc.tile_pool(name="pc_e", bufs=3))
            pc_o = sc_.enter_context(tc.tile_pool(name="pc_o", bufs=3))
            pc_sc = sc_.enter_context(
                tc.tile_pool(name="pc_sc", bufs=4, space="PSUM"))
            pc_acc = sc_.enter_context(
                tc.tile_pool(name="pc_acc", bufs=1, space="PSUM"))
            for h in range(NH):
                for qt in range(RT):
                    qsl = bass.ts(qt, 512)
                    sums = pc_acc.tile([128, 512], F32, tag="sums", name="sums")
                    ctx0 = pc_acc.tile([128, 512], F32, tag="ctx0", name="ctx0")
                    ctx1 = pc_acc.tile([128, 512], F32, tag="ctx1", name="ctx1")
                    # software pipeline: scores(kp) on PE overlap exp(kp-1)
                    e2s = [None] * KC
                    def emit_scores(kp):
                        sc = pc_sc.tile([128, 512], F32, tag="sc", name="sc")
                        nc.tensor.matmul(
                            sc, ksb[:, 2 * h:2 * h + 2, kp * 128:(kp + 1) * 128],
                            q8_sb[:, 2 * h:2 * h + 2, qsl],
                            start=True, stop=True, perf_mode=DR)
                        return sc

                    def emit_exp(kp, scs):
                        j = kp % 2
                        if j == 0:
                            e2s[kp // 2] = pc_e.tile([128, 2, 512], F8,
                                                     tag="e2", name="e2")
                        nc.scalar.activation(out=e2s[kp // 2][:, j], in_=scs,
                                             func=AF.Exp, scale=1.0 / 1024.0)

                    def emit_acc(pp):  # pair index 0..7 over e2s
                        e2 = e2s[pp]
                        st, sp = (pp == 0), (pp == KC // 2 - 1)
                        nc.tensor.matmul(sums, ones8, e2, start=st, stop=sp,
                                         perf_mode=DR)
                        nc.tensor.matmul(
                            ctx0, vsb[:, 2 * pp:2 * pp + 2,
                                      h * HD:h * HD + 128],
                            e2, start=st, stop=sp, perf_mode=DR)
                        nc.tensor.matmul(
                            ctx1, vsb[:, 2 * pp:2 * pp + 2,
                                      h * HD + 128:h * HD + 256],
                            e2, start=st, stop=sp, perf_mode=DR)

                    # PE order: sc0 sc1 sc2 sc3 acc0 sc4 sc5 acc1 ... acc6 acc7
                    # so exp(kp) always has 2+ score-matmuls of PE work ahead.
                    for kp in range(KC):
                        emit_exp(kp, emit_scores(kp))
                        if kp % 2 == 1 and kp >= 3:
                            emit_acc(kp // 2 - 1)
                    emit_acc(KC // 2 - 1)

                    rec = pc_o.tile([128, 512], F32, tag="rec", name="rec")
                    nc.vector.reciprocal(out=rec, in_=sums)
                    for j, ctx in enumerate((ctx0, ctx1)):
                        t = pc_o.tile([128, 512], F32, tag="ctmp", name="ctmp")
                        nc.vector.tensor_mul(t, ctx, rec)
                        nc.vector.tensor_scalar_add(
                            ctxb_sb[:, 2 * h + j, qsl], t,
                            bv_sb[:, 2 * h + j:2 * h + j + 1])
                        nc.gpsimd.tensor_copy(
                            out=ctx8_sb[:, 2 * h + j, qsl],
                            in_=ctxb_sb[:, 2 * h + j, qsl])

        def cat_f(hc):   # f32r/bf16 cat chunk for h1
            return xt_sb[:, hc, :] if hc < HC else ctxb_sb[:, hc - HC, :]

        def cat8(hc):    # fp8 cat chunk for gate
            return x8_sb[:, hc, :] if hc < HC else ctx8_sb[:, hc - HC, :]

        # =========== Phase D0: gate (fp8 DoubleRow) ===========
        sigb_sb = pDlong.tile([128, HC, R], BF16)     # 2 MB
        with ExitStack() as sd0:
            pd0_w = sd0.enter_context(tc.tile_pool(name="pd0_w", bufs=2))
            pd0_ps = sd0.enter_context(
                tc.tile_pool(name="pd0_ps", bufs=2, space="PSUM"))
            for oc in range(HC):
                gws = pd0_w.tile([128, H2C, 128], F8, tag="gw", name="gws")
                nc.sync.dma_start(out=gws, in_=d_gw[oc])
                for rt in range(RT):
                    sl = bass.ts(rt, 512)
                    ps = pd0_ps.tile([128, 512], F32, tag="gps", name="gps")
                    for p in range(8):
                        c = 2 * p
                        rhs0 = cat8(c)
                        rhs1 = cat8(c + 1)
                        # pairs never straddle the x8/ctx8 boundary (8|c)
                        src = x8_sb if c < HC else ctx8_sb
                        cc = c if c < HC else c - HC
                        nc.tensor.matmul(ps, gws[:, c:c + 2, :],
                                         src[:, cc:cc + 2, sl],
                                         start=(p == 0), stop=(p == 7),
                                         perf_mode=DR)
                    nc.scalar.activation(out=sigb_sb[:, oc, sl], in_=ps,
                                         func=AF.Sigmoid, scale=1.0 / WS,
                                         bias=gb_sb[:, oc:oc + 1])

        pBC.release()   # frees ksb/vsb/q8/x8/ctx8 (5+2 MB)

        # =========== Phase D1: h1 = cat @ w1.T + b1, with inline stats =====
        h1_sb = pDlong.tile([128, H2C, R], BF16)      # 4 MB
        mu2_f = pDlong.tile([128, R], F32)
        rstd2_f = pDlong.tile([128, R], F32)
        with ExitStack() as sd1:
            pd1_w = sd1.enter_context(tc.tile_pool(name="pd1_w", bufs=3))
            pd1_sq = sd1.enter_context(tc.tile_pool(name="pd1_sq", bufs=3))
            pd1_ps = sd1.enter_context(
                tc.tile_pool(name="pd1_ps", bufs=3, space="PSUM"))
            pd1_st = sd1.enter_context(
                tc.tile_pool(name="pd1_st", bufs=1, space="PSUM"))
            mu2_ps = [pd1_st.tile([128, 512], F32, tag=f"mu{i}", name=f"mu{i}")
                      for i in range(RT)]
            ms2_ps = [pd1_st.tile([128, 512], F32, tag=f"ms{i}", name=f"ms{i}")
                      for i in range(RT)]
            for oc2 in range(H2C):
                w1s = pd1_w.tile([128, H2C, 128], F32R, tag="w1", name="w1s")
                nc.sync.dma_start(out=w1s, in_=d_w1[oc2])
                for rt in range(RT):
                    sl = bass.ts(rt, 512)
                    ps = pd1_ps.tile([128, 512], F32, tag="h1ps", name="h1ps")
                    for hc in range(H2C):
                        nc.tensor.matmul(ps, w1s[:, hc, :], cat_f(hc)[:, sl],
                                         start=(hc == 0), stop=(hc == H2C - 1))
                    nc.scalar.activation(out=h1_sb[:, oc2, sl], in_=ps,
                                         func=AF.Identity,
                                         bias=b1_sb[:, oc2:oc2 + 1])
                    sq = pd1_sq.tile([128, 512], BF16, tag="sq", name="sq")
                    nc.vector.tensor_mul(sq, h1_sb[:, oc2, sl],
                                         h1_sb[:, oc2, sl])
                    nc.tensor.matmul(mu2_ps[rt], ones2kb, h1_sb[:, oc2, sl],
                                     start=(oc2 == 0), stop=(oc2 == H2C - 1))
                    nc.tensor.matmul(ms2_ps[rt], ones2kb, sq,
                                     start=(oc2 == 0), stop=(oc2 == H2C - 1))
            for rt in range(RT):
                sl = bass.ts(rt, 512)
                nc.scalar.activation(out=mu2_f[:, sl], in_=mu2_ps[rt],
                                     func=AF.Copy)
                var = pd1_sq.tile([128, 512], F32, tag="var", name="var")
                nc.vector.tensor_mul(var, mu2_f[:, sl], mu2_f[:, sl])
                nc.vector.tensor_sub(var, ms2_ps[rt], var)
                nc.scalar.activation(out=var, in_=var, func=AF.Ln,
                                     bias=eps_t, scale=1.0)
                nc.scalar.activation(out=rstd2_f[:, sl], in_=var,
                                     func=AF.Exp, scale=-0.5)

        # =========== Phase D2: layernorm apply + gelu (in place) ===========
        with ExitStack() as sd2:
            pd2 = sd2.enter_context(tc.tile_pool(name="pd2", bufs=3))
            for oc2 in range(H2C):
                t1 = pd2.tile([128, R], F32, tag="t1", name="t1")
                nc.gpsimd.tensor_sub(t1, h1_sb[:, oc2, :], mu2_f)
                nc.vector.scalar_tensor_tensor(
                    out=t1, in0=t1, scalar=ilg_sb[:, oc2:oc2 + 1],
                    in1=rstd2_f, op0=OP.mult, op1=OP.mult)
                nc.scalar.activation(out=h1_sb[:, oc2, :], in_=t1,
                                     func=AF.Gelu,
                                     bias=ilb_sb[:, oc2:oc2 + 1])

        # ====== Phase D3+D4: integ, y = x + gate*integ, final layernorm =====
        with ExitStack() as sd3:
            pd3_big = sd3.enter_context(tc.tile_pool(name="pd3_big", bufs=1))
            w2_all = pd3_big.tile([128, HC, H2C, 128], BF16)   # 4 MB
            for oc in range(HC):
                nc.sync.dma_start(out=w2_all[:, oc], in_=d_w2[oc])

            l2g_bc = pd3_big.tile([128, H], F32)
            nc.sync.dma_start(
                out=l2g_bc,
                in_=d_l2g[:].unsqueeze(0).partition_broadcast(128).squeeze(1))
            l2b_bc = pd3_big.tile([128, H], F32)
            nc.sync.dma_start(
                out=l2b_bc,
                in_=d_l2b[:].unsqueeze(0).partition_broadcast(128).squeeze(1))
            pd3_o = sd3.enter_context(tc.tile_pool(name="pd3_o", bufs=2))
            pd3_ps = sd3.enter_context(
                tc.tile_pool(name="pd3_ps", bufs=2, space="PSUM"))
            pd3_st = sd3.enter_context(
                tc.tile_pool(name="pd3_st", bufs=1, space="PSUM"))
            pd3_tp = sd3.enter_context(
                tc.tile_pool(name="pd3_tp", bufs=2, space="PSUM"))
            for rt in range(RT):
                sl = bass.ts(rt, 512)
                yt = pd3_big.tile([128, HC, 512], F32R, tag="yt", name="yt")
                for oc in range(HC):
                    ps = pd3_ps.tile([128, 512], F32, tag="w2ps", name="w2ps")
                    for hc in range(H2C):
                        nc.tensor.matmul(ps, w2_all[:, oc, hc, :],
                                         h1_sb[:, hc, sl],
                                         start=(hc == 0), stop=(hc == H2C - 1))
                    ytmp = pd3_o.tile([128, 512], F32, tag="ytmp", name="ytmp")
                    nc.vector.scalar_tensor_tensor(
                        out=ytmp, in0=ps, scalar=b2_sb[:, oc:oc + 1],
                        in1=sigb_sb[:, oc, sl], op0=OP.add, op1=OP.mult)
                    nc.gpsimd.tensor_add(yt[:, oc, :], ytmp,
                                         xt_sb[:, oc, sl].bitcast(F32))
                # final layernorm stats for this row tile (feature-major)
                muy = pd3_st.tile([128, 512], F32, tag="muy", name="muy")
                msy = pd3_st.tile([128, 512], F32, tag="msy", name="msy")
                for oc in range(HC):
                    sqy = pd3_o.tile([128, 512], F32R, tag="sqy", name="sqy")
                    nc.vector.tensor_mul(sqy, yt[:, oc, :].bitcast(F32),
                                         yt[:, oc, :].bitcast(F32))
                    nc.tensor.matmul(muy, ones1k, yt[:, oc, :],
                                     start=(oc == 0), stop=(oc == HC - 1))
                    nc.tensor.matmul(msy, ones1k, sqy,
                                     start=(oc == 0), stop=(oc == HC - 1))
                muy_f = pd3_o.tile([128, 512], F32, tag="muyf", name="muyf", bufs=1)
                nc.scalar.activation(out=muy_f, in_=muy, func=AF.Copy)
                var = pd3_o.tile([128, 512], F32, tag="vary", name="vary", bufs=1)
                nc.vector.tensor_mul(var, muy_f, muy_f)
                nc.vector.tensor_sub(var, msy, var)
                nc.scalar.activation(out=var, in_=var, func=AF.Ln,
                                     bias=eps_t, scale=1.0)
                rstdy = pd3_o.tile([128, 512], F32, tag="rsty", name="rstdy", bufs=1)
                nc.scalar.activation(out=rstdy, in_=var, func=AF.Exp,
                                     scale=-0.5)
                # normalize in feature-major (per-column mu/rstd), then
                # transpose and apply gamma/beta row-major
                for oc in range(HC):
                    t = pd3_o.tile([128, 512], F32, tag="ynt", name="ynt")
                    nc.vector.tensor_sub(t, yt[:, oc, :].bitcast(F32), muy_f)
                    nc.gpsimd.tensor_mul(yt[:, oc, :], t, rstdy)
                for j in range(4):
                    rc = rt * 4 + j
                    tp = pd3_tp.tile([128, H], F32R, tag="tp", name="tp")
                    for oc in range(HC):
                        nc.tensor.transpose(
                            tp[:, oc * 128:(oc + 1) * 128],
                            yt[:, oc, j * 128:(j + 1) * 128], ident)
                    yr = pd3_o.tile([128, H], F32, tag="yr", name="yr")
                    nc.vector.tensor_mul(yr, tp.bitcast(F32), l2g_bc)
                    nc.gpsimd.tensor_add(yr, yr, l2b_bc)
                    nc.sync.dma_start(out=d_out[rc * 128:(rc + 1) * 128, :],
                                      in_=yr)

    nc.compile()
    return nc


_NC_CACHE = []


def _get_nc():
    if not _NC_CACHE:
        _NC_CACHE.append(build_program())
    return _NC_CACHE[0]


def _fp8(a, scale=1.0):
    return np.asarray(np.asarray(a, np.float32) * scale).astype(
        ml_dtypes.float8_e4m3)


def _chunk(w_t, dtype, scale=1.0):
    # [IN, OUT] -> [OUT//128, 128, IN//128, 128] contiguous per-partition
    inn, out = w_t.shape
    r = (w_t * scale).reshape(inn // 128, 128, out // 128, 128)
    r = r.transpose(2, 1, 0, 3)
    return np.ascontiguousarray(r.astype(dtype))


def kernel(query_hidden, mem_keys, importance, recency, access_count,
           Wq, bq, in_w, in_b, out_w, out_b, gate_w, gate_b,
           int_w1, int_b1, int_ln_g, int_ln_b, int_w2, int_b2,
           ln1_g, ln1_b, ln2_g, ln2_b, sel_params, top_k):
    np32 = lambda a: np.asarray(a, dtype=np.float32)
    query_hidden = np32(query_hidden)
    mem_keys = np32(mem_keys)
    top_k = int(top_k)
    assert top_k == K, f"kernel compiled for top_k={K}, got {top_k}"

    # HTPS selection (host): softmax-weighted score, top-k set, gather.
    sp = np32(sel_params)
    w = np.exp(sp - sp.max())
    w = w / w.sum()
    acc = np32(access_count)
    sel = w[0] * np32(importance) + w[1] * np32(recency) + w[2] * (acc / acc.max())
    idx = np.argpartition(-sel, top_k - 1)[:top_k]
    mem = mem_keys[idx]                                  # [K, H]

    # memory layernorm on host (ln1 gamma/beta folded into wk/wv below)
    mu = mem.mean(1, keepdims=True)
    var = ((mem - mu) ** 2).mean(1, keepdims=True)
    mem_n = (mem - mu) / np.sqrt(var + EPS)
    mn8_t = np.ascontiguousarray(_fp8(mem_n.T))          # [H, K] fp8

    in_w = np32(in_w)
    in_b = np32(in_b)
    wq, wk, wv = in_w[:H], in_w[H:2 * H], in_w[2 * H:]
    bqi, bki, bvi = in_b[:H], in_b[H:2 * H], in_b[2 * H:]
    wc = wq @ np32(Wq)                                   # fused query proj
    bc = wq @ np32(bq) + bqi

    g1 = np32(ln1_g)
    b1v = np32(ln1_b)
    bki = bki + wk @ b1v
    bvi = bvi + wv @ b1v
    wk = wk * g1[None, :]
    wv = wv * g1[None, :]

    out_w = np32(out_w)
    out_b = np32(out_b)
    gate_w = np32(gate_w)
    int_w1 = np32(int_w1)
    gwx, gwa = gate_w[:, :H], gate_w[:, H:]
    w1x, w1a = int_w1[:, :H], int_w1[:, H:]
    gate_b_f = np32(gate_b) + gwa @ out_b
    int_b1_f = np32(int_b1) + w1a @ out_b

    T = lambda a: np.ascontiguousarray(np32(a).T)
    gw_t = np.concatenate([gwx.T, (gwa @ out_w).T], axis=0)   # [2H, H]
    w1_t = np.concatenate([w1x.T, (w1a @ out_w).T], axis=0)   # [2H, 2H]

    # wv8: [128, HC, H] = wv.T reshaped (in-chunk-major partitions)
    wv_t = (T(wv) * WS).reshape(HC, 128, H).transpose(1, 0, 2)
    common = {
        "mn8_t": mn8_t,
        "wc8": _chunk(T(wc), ml_dtypes.float8_e4m3, WS),
        "wk8": _chunk(T(wk), ml_dtypes.float8_e4m3, WS),
        "wv8": np.ascontiguousarray(wv_t.astype(ml_dtypes.float8_e4m3)),
        "gw8": _chunk(gw_t, ml_dtypes.float8_e4m3, WS),
        "w1b": _chunk(w1_t, np.float32),
        "w2b": _chunk(T(np32(int_w2)), ml_dtypes.bfloat16),
        "bc8": QS * bc, "bk8": QS * bki, "bv": bvi,
        "gate_b": gate_b_f, "int_b1": int_b1_f, "int_b2": np32(int_b2),
        "iln_g": np32(int_ln_g), "iln_b": np32(int_ln_b),
        "ln2_g": np32(ln2_g), "ln2_b": np32(ln2_b),
    }
    X = query_hidden.reshape(B * S, H)
    in_maps = []
    for c in range(N_CORES):
        m = dict(common)
        xc_t = np.ascontiguousarray(X[c * R:(c + 1) * R].T)
        m["x_t"] = xc_t
        m["x8_t"] = np.ascontiguousarray(_fp8(xc_t))
        in_maps.append(m)

    nc = _get_nc()
    res = run_bass_kernel_spmd(nc, in_maps, core_ids=list(range(N_CORES)))
    out = np.empty((B * S, H), dtype=np.float32)
    for c in range(N_CORES):
        out[c * R:(c + 1) * R] = res.results[c]["out"]
    return out.reshape(B, S, H)


# revision 17
# speedup vs baseline: 1.4772x; 1.0395x over previous
"""MemoryRetriever kernel for 8x Trainium2 NeuronCores.

Data-parallel over the B*S=8192 query rows (1024 rows/core); the selected
memory bank and all weights are replicated.

Precision plan (validated vs reference on CPU and HW):
  - K/V/Q projections, attention scores/softmax/context, the sigmoid gate,
    and h1 = cat @ w1 run in fp8e4 (e4m3) with DoubleRow matmuls (two
    128-row contraction chunks per instruction, 2x fp32r throughput on HW).
    fp8 weights are prescaled x64 on host; Q/K activations stored x8.
  - integ = gelu(h1n) @ w2 runs in bf16 (fp8 there exceeds tolerance).
  Expected rel err ~1.5e-2 < 2e-2 gate (CPU sim of this exact config).

Host-side fusions (exact up to rounding): fused query projection
(wq_in@Wq), memory layernorm (device gets mem_n as fp8 directly), ln1
gamma/beta folded into wk/wv, attn_out = ctx@out_w.T + out_b folded into
the gate/integration weights so cat == [x; ctx] on device.

Device layout is feature-major: [feature chunk of 128 partitions, rows].
K and V stay SBUF-resident (fp8, 2 MB each) - no DRAM roundtrip.
The final layernorm folds ln2 gamma/beta into the feature-major
normalize (per-partition scale/bias), then PE-transposes to row-major.
"""

import sys
from contextlib import ExitStack

if "/opt/trn_rl_repo" not in sys.path:
    sys.path.insert(0, "/opt/trn_rl_repo")

import numpy as np
import ml_dtypes

import concourse.bass as bass
import concourse.mybir as mybir
import concourse.tile as tile
from concourse import bacc
from concourse.bass_utils import run_bass_kernel_spmd
from concourse.masks import make_identity

F32 = mybir.dt.float32
F32R = mybir.dt.float32r
BF16 = mybir.dt.bfloat16
F8 = mybir.dt.float8e4
AF = mybir.ActivationFunctionType
OP = mybir.AluOpType
DR = mybir.MatmulPerfMode.DoubleRow

H = 1024
NH = 4
HD = H // NH          # 256
K = 2048              # top_k
B, S = 4, 2048
N_CORES = 8
R = (B * S) // N_CORES  # 1024 rows per core
EPS = 1e-5
H2 = 2 * H            # 2048

HC = H // 128         # 8 feature chunks
H2C = H2 // 128       # 16
KC = K // 128         # 16 key chunks
RT = R // 512         # 2 row tiles of 512
KT4 = K // 512        # 4 key tiles of 512
WS = 64.0             # fp8 weight prescale
QS = 8.0              # fp8 Q/K activation prescale


def build_program():
    nc = bacc.Bacc("TRN2", target_bir_lowering=False)

    d_xt = nc.declare_dram_parameter("x_t", [H, R], F32R, isOutput=False)
    d_x8 = nc.declare_dram_parameter("x8_t", [H, R], F8, isOutput=False)
    d_mn8 = nc.declare_dram_parameter("mn8_t", [H, K], F8, isOutput=False)
    d_wc = nc.declare_dram_parameter("wc8", [HC, 128, HC, 128], F8,
                                     isOutput=False)
    d_wk = nc.declare_dram_parameter("wk8", [HC, 128, HC, 128], F8,
                                     isOutput=False)
    d_wv = nc.declare_dram_parameter("wv8", [128, HC, H], F8, isOutput=False)
    d_gw = nc.declare_dram_parameter("gw8", [HC, 128, H2C, 128], F8,
                                     isOutput=False)
    d_w1 = nc.declare_dram_parameter("w1b", [H2C, 128, H2C, 128], F8,
                                     isOutput=False)
    d_w2 = nc.declare_dram_parameter("w2b", [HC, 128, H2C, 128], BF16,
                                     isOutput=False)
    d_bc = nc.declare_dram_parameter("bc8", [H], F32, isOutput=False)
    d_bk = nc.declare_dram_parameter("bk8", [H], F32, isOutput=False)
    d_bv = nc.declare_dram_parameter("bv", [H], F32, isOutput=False)
    d_gb = nc.declare_dram_parameter("gate_b", [H], F32, isOutput=False)
    d_b1 = nc.declare_dram_parameter("int_b1", [H2], F32, isOutput=False)
    d_b2 = nc.declare_dram_parameter("int_b2", [H], F32, isOutput=False)
    d_ilg = nc.declare_dram_parameter("iln_g", [H2], F32, isOutput=False)
    d_ilb = nc.declare_dram_parameter("iln_b", [H2], F32, isOutput=False)
    d_l2g = nc.declare_dram_parameter("ln2_g", [H], F32, isOutput=False)
    d_l2b = nc.declare_dram_parameter("ln2_b", [H], F32, isOutput=False)
    d_out = nc.declare_dram_parameter("out", [R, H], F32, isOutput=True)

    with tile.TileContext(nc) as tc, ExitStack() as top:
        singles = top.enter_context(tc.tile_pool(name="singles", bufs=1))

        ident_f = singles.tile([128, 128], F32)
        make_identity(nc, ident_f)
        ident = singles.tile([128, 128], F32R)
        nc.scalar.activation(out=ident, in_=ident_f, func=AF.Copy)
        scratch1 = singles.tile([128, 128], F32)
        nc.vector.memset(scratch1, 1.0)
        ones8 = singles.tile([128, 2, 128], F8)
        nc.scalar.activation(out=ones8[:, 0], in_=scratch1, func=AF.Copy)
        nc.scalar.activation(out=ones8[:, 1], in_=scratch1, func=AF.Copy)
        ones1k = singles.tile([128, 128], F32R)
        nc.scalar.activation(out=ones1k, in_=scratch1, func=AF.Copy,
                             scale=1.0 / 1024.0)
        ones2kb = singles.tile([128, 128], BF16)
        nc.scalar.activation(out=ones2kb, in_=scratch1, func=AF.Copy,
                             scale=1.0 / 2048.0)
        eps_t = singles.tile([128, 1], F32)
        nc.vector.memset(eps_t, EPS)

        def load_pp(vec, n, nm):  # [n*128] dram vector -> [128, n]
            t = singles.tile([128, n], F32, tag=f"pp_{nm}", name=f"pp_{nm}")
            nc.sync.dma_start(out=t, in_=vec[:].rearrange("(c p) -> p c", p=128))
            return t

        bc8_sb = load_pp(d_bc, HC, "bc")
        bk8_sb = load_pp(d_bk, HC, "bk")
        bv_sb = load_pp(d_bv, HC, "bv")
        gb_sb = load_pp(d_gb, HC, "gb")
        b1_sb = load_pp(d_b1, H2C, "b1")
        b2_sb = load_pp(d_b2, HC, "b2")
        ilg_sb = load_pp(d_ilg, H2C, "ilg")
        ilb_sb = load_pp(d_ilb, H2C, "ilb")
        l2g_sb = load_pp(d_l2g, HC, "l2g")
        l2b_sb = load_pp(d_l2b, HC, "l2b")
        xt_sb = singles.tile([128, HC, R], F32R)      # 4 MB, residual
        nc.sync.dma_start(
            out=xt_sb, in_=d_xt[:].rearrange("(c p) r -> p c r", p=128))

        # pool lifetimes (stack/LIFO): pDlong outlives pBC outlives pA
        pDlong = top.enter_context(tc.tile_pool(name="pDlong", bufs=1))
        pBC = tc.alloc_tile_pool(name="pBC", bufs=1)  # released after D1

        x8_sb = pBC.tile([128, HC, R], F8)            # 1 MB
        nc.sync.dma_start(
            out=x8_sb, in_=d_x8[:].rearrange("(c p) r -> p c r", p=128))
        ksb = pBC.tile([128, HC, K], F8)              # 2 MB: K^T, 8*K_true
        vsb = pBC.tile([128, KC, H], F8)              # 2 MB: V rows
        q8_sb = pBC.tile([128, HC, R], F8)            # 1 MB: 8*Q
        ctx8_sb = pBC.tile([128, HC, R], F8)          # 1 MB: fp8 ctx (+bv)

        def cat8(c):   # fp8 cat chunk c in [0, 16)
            return x8_sb if c < HC else ctx8_sb

        # =========== Phase A: K/V projections (fp8 DoubleRow) ===========
        with ExitStack() as sa:
            pa_in = sa.enter_context(tc.tile_pool(name="pa_in", bufs=1))
            mn8_sb = pa_in.tile([128, HC, K], F8)     # 2 MB, layernormed mem
            nc.sync.dma_start(
                out=mn8_sb, in_=d_mn8[:].rearrange("(c p) k -> p c k", p=128))
            wv8_sb = pa_in.tile([128, HC, H], F8)     # 1 MB
            nc.sync.dma_start(out=wv8_sb, in_=d_wv[:])
            pa_w = sa.enter_context(tc.tile_pool(name="pa_w", bufs=3))
            pa_ps = sa.enter_context(
                tc.tile_pool(name="pa_ps", bufs=2, space="PSUM"))
            for oc in range(HC):
                wks = pa_w.tile([128, HC, 128], F8, tag="wk", name="wks")
                nc.sync.dma_start(out=wks, in_=d_wk[oc])
                for kt in range(KT4):
                    sl = bass.ts(kt, 512)
                    ps = pa_ps.tile([128, 512], F32, tag="kps", name="kps")
                    for p in range(4):
                        nc.tensor.matmul(ps, wks[:, 2 * p:2 * p + 2, :],
                                         mn8_sb[:, 2 * p:2 * p + 2, sl],
                                         start=(p == 0), stop=(p == 3),
                                         perf_mode=DR)
                    nc.scalar.activation(out=ksb[:, oc, sl], in_=ps,
                                         func=AF.Identity, scale=1.0 / 8.0,
                                         bias=bk8_sb[:, oc:oc + 1])
            for kc in range(KC):
                ps = pa_ps.tile([128, H], F32, tag="vps", name="vps")
                for half in range(2):
                    hsl = bass.ts(half, 512)
                    for p in range(4):
                        nc.tensor.matmul(
                            ps[:, hsl],
                            mn8_sb[:, 2 * p:2 * p + 2, kc * 128:(kc + 1) * 128],
                            wv8_sb[:, 2 * p:2 * p + 2, hsl],
                            start=(p == 0), stop=(p == 3), perf_mode=DR)
                # V stored unscaled: psum = 64*V -> scale 1/64
                nc.vector.tensor_scalar_mul(vsb[:, kc, :], ps, 1.0 / WS)

        # =========== Phase B: query projection (fp8 DoubleRow) ===========
        with ExitStack() as sb_:
            pb_w = sb_.enter_context(tc.tile_pool(name="pb_w", bufs=3))
            pb_ps = sb_.enter_context(
                tc.tile_pool(name="pb_ps", bufs=2, space="PSUM"))
            for oc in range(HC):
                wcs = pb_w.tile([128, HC, 128], F8, tag="wc", name="wcs")
                nc.sync.dma_start(out=wcs, in_=d_wc[oc])
                ps = pb_ps.tile([128, R], F32, tag="qps", name="qps")
                for rt in range(RT):
                    sl = bass.ts(rt, 512)
                    for p in range(4):
                        nc.tensor.matmul(ps[:, sl],
                                         wcs[:, 2 * p:2 * p + 2, :],
                                         x8_sb[:, 2 * p:2 * p + 2, sl],
                                         start=(p == 0), stop=(p == 3),
                                         perf_mode=DR)
                nc.scalar.activation(out=q8_sb[:, oc, :], in_=ps,
                                     func=AF.Identity, scale=1.0 / 8.0,
                                     bias=bc8_sb[:, oc:oc + 1])

        # =========== Phase C: attention (fp8 DoubleRow + exp pipeline) =====
        with ExitStack() as sc_:
            pc_e = sc_.enter_context(tc.tile_pool(name="pc_e", bufs=3))
            pc_o = sc_.enter_context(tc.tile_pool(name="pc_o", bufs=3))
            pc_sc = sc_.enter_context(
                tc.tile_pool(name="pc_sc", bufs=2, space="PSUM"))
            pc_acc = sc_.enter_context(
                tc.tile_pool(name="pc_acc", bufs=1, space="PSUM"))
            for h in range(NH):
                for qt in range(RT):
                    qsl = bass.ts(qt, 512)
                    sums = pc_acc.tile([128, 512], F32, tag="sums", name="sums")
                    ctx0 = pc_acc.tile([128, 512], F32, tag="ctx0", name="ctx0")
                    ctx1 = pc_acc.tile([128, 512], F32, tag="ctx1", name="ctx1")
                    e2s = [None] * (KC // 2)

                    def emit_scores_exp(pp):
                        sc = pc_sc.tile([128, 2, 512], F32, tag="sc", name="sc")
                        for j in range(2):
                            kp = 2 * pp + j
                            nc.tensor.matmul(
                                sc[:, j],
                                ksb[:, 2 * h:2 * h + 2,
                                    kp * 128:(kp + 1) * 128],
                                q8_sb[:, 2 * h:2 * h + 2, qsl],
                                start=True, stop=True, perf_mode=DR)
                        e2s[pp] = pc_e.tile([128, 2, 512], F8, tag="e2",
                                            name="e2")
                        nc.scalar.activation(out=e2s[pp], in_=sc, func=AF.Exp,
                                             scale=1.0 / 1024.0)

                    def emit_acc(pp):
                        e2 = e2s[pp]
                        st, sp = (pp == 0), (pp == KC // 2 - 1)
                        nc.tensor.matmul(sums, ones8, e2, start=st, stop=sp,
                                         perf_mode=DR)
                        nc.tensor.matmul(
                            ctx0, vsb[:, 2 * pp:2 * pp + 2,
                                      h * HD:h * HD + 128],
                            e2, start=st, stop=sp, perf_mode=DR)
                        nc.tensor.matmul(
                            ctx1, vsb[:, 2 * pp:2 * pp + 2,
                                      h * HD + 128:h * HD + 256],
                            e2, start=st, stop=sp, perf_mode=DR)

                    # PE order: sc(0) sc(1) acc(0) sc(2) acc(1) ... acc(7)
                    # so each exp has a pair of score-matmuls running behind it
                    for pp in range(KC // 2):
                        emit_scores_exp(pp)
                        if pp >= 1:
                            emit_acc(pp - 1)
                    emit_acc(KC // 2 - 1)

                    rec = pc_o.tile([128, 512], F32, tag="rec", name="rec")
                    nc.vector.reciprocal_approx_fast(out=rec, in_=sums)
                    for j, ctx in enumerate((ctx0, ctx1)):
                        t = pc_o.tile([128, 512], F32, tag="ctmp", name="ctmp")
                        nc.vector.tensor_mul(t, ctx, rec)
                        nc.vector.tensor_scalar_add(
                            ctx8_sb[:, 2 * h + j, qsl], t,
                            bv_sb[:, 2 * h + j:2 * h + j + 1])

        # =========== Phase D0: gate (fp8 DoubleRow) ===========
        sigb_sb = pDlong.tile([128, HC, R], BF16)     # 2 MB
        with ExitStack() as sd0:
            pd0_w = sd0.enter_context(tc.tile_pool(name="pd0_w", bufs=2))
            pd0_ps = sd0.enter_context(
                tc.tile_pool(name="pd0_ps", bufs=2, space="PSUM"))
            for oc in range(HC):
                gws = pd0_w.tile([128, H2C, 128], F8, tag="gw", name="gws")
                nc.sync.dma_start(out=gws, in_=d_gw[oc])
                ps = pd0_ps.tile([128, R], F32, tag="gps", name="gps")
                for rt in range(RT):
                    sl = bass.ts(rt, 512)
                    for p in range(8):
                        c = 2 * p
                        src = cat8(c)
                        cc = c if c < HC else c - HC
                        nc.tensor.matmul(ps[:, sl], gws[:, c:c + 2, :],
                                         src[:, cc:cc + 2, sl],
                                         start=(p == 0), stop=(p == 7),
                                         perf_mode=DR)
                nc.scalar.activation(out=sigb_sb[:, oc, :], in_=ps,
                                     func=AF.Sigmoid, scale=1.0 / WS,
                                     bias=gb_sb[:, oc:oc + 1])

        # ======= Phase D1: h1 = cat8 @ w1.T + b1 (fp8 DR), inline stats ====
        h1_sb = pDlong.tile([128, H2C, R], BF16)      # 4 MB
        mu2_f = pDlong.tile([128, R], F32)
        rstd2_f = pDlong.tile([128, R], F32)
        with ExitStack() as sd1:
            pd1_w = sd1.enter_context(tc.tile_pool(name="pd1_w", bufs=3))
            pd1_sq = sd1.enter_context(tc.tile_pool(name="pd1_sq", bufs=3))
            pd1_ps = sd1.enter_context(
                tc.tile_pool(name="pd1_ps", bufs=2, space="PSUM"))
            pd1_st = sd1.enter_context(
                tc.tile_pool(name="pd1_st", bufs=1, space="PSUM"))
            mu2_ps = [pd1_st.tile([128, 512], F32, tag=f"mu{i}", name=f"mu{i}")
                      for i in range(RT)]
            ms2_ps = [pd1_st.tile([128, 512], F32, tag=f"ms{i}", name=f"ms{i}")
                      for i in range(RT)]
            for oc2 in range(H2C):
                w1s = pd1_w.tile([128, H2C, 128], F8, tag="w1", name="w1s")
                nc.sync.dma_start(out=w1s, in_=d_w1[oc2])
                ps = pd1_ps.tile([128, R], F32, tag="h1ps", name="h1ps")
                for rt in range(RT):
                    sl = bass.ts(rt, 512)
                    for p in range(8):
                        c = 2 * p
                        src = cat8(c)
                        cc = c if c < HC else c - HC
                        nc.tensor.matmul(ps[:, sl], w1s[:, c:c + 2, :],
                                         src[:, cc:cc + 2, sl],
                                         start=(p == 0), stop=(p == 7),
                                         perf_mode=DR)
                nc.scalar.activation(out=h1_sb[:, oc2, :], in_=ps,
                                     func=AF.Identity, scale=1.0 / WS,
                                     bias=b1_sb[:, oc2:oc2 + 1])
                sq = pd1_sq.tile([128, R], BF16, tag="sq", name="sq")
                nc.vector.tensor_mul(sq, h1_sb[:, oc2, :], h1_sb[:, oc2, :])
                for rt in range(RT):
                    sl = bass.ts(rt, 512)
                    nc.tensor.matmul(mu2_ps[rt], ones2kb, h1_sb[:, oc2, sl],
                                     start=(oc2 == 0), stop=(oc2 == H2C - 1))
                    nc.tensor.matmul(ms2_ps[rt], ones2kb, sq[:, sl],
                                     start=(oc2 == 0), stop=(oc2 == H2C - 1))
            for rt in range(RT):
                sl = bass.ts(rt, 512)
                nc.scalar.activation(out=mu2_f[:, sl], in_=mu2_ps[rt],
                                     func=AF.Copy)
                var = pd1_sq.tile([128, 512], F32, tag="var", name="var")
                nc.vector.tensor_mul(var, mu2_f[:, sl], mu2_f[:, sl])
                nc.vector.tensor_sub(var, ms2_ps[rt], var)
                nc.scalar.activation(out=var, in_=var, func=AF.Ln,
                                     bias=eps_t, scale=1.0)
                nc.scalar.activation(out=rstd2_f[:, sl], in_=var,
                                     func=AF.Exp, scale=-0.5)

        pBC.release()   # frees ksb/vsb/q8/x8/ctx8 (7 MB)

        # =========== Phase D2: layernorm apply + gelu (in place) ===========
        with ExitStack() as sd2:
            pd2 = sd2.enter_context(tc.tile_pool(name="pd2", bufs=3))
            for oc2 in range(H2C):
                t1 = pd2.tile([128, R], F32, tag="t1", name="t1")
                if oc2 % 2 == 0:
                    nc.gpsimd.tensor_sub(t1, h1_sb[:, oc2, :], mu2_f)
                else:
                    nc.vector.tensor_sub(t1, h1_sb[:, oc2, :], mu2_f)
                nc.vector.scalar_tensor_tensor(
                    out=t1, in0=t1, scalar=ilg_sb[:, oc2:oc2 + 1],
                    in1=rstd2_f, op0=OP.mult, op1=OP.mult)
                nc.scalar.activation(out=h1_sb[:, oc2, :], in_=t1,
                                     func=AF.Gelu,
                                     bias=ilb_sb[:, oc2:oc2 + 1])

        # ====== Phase D3: integ (bf16), y = x + gate*integ ===========
        with ExitStack() as sd3:
            pd3_big = sd3.enter_context(tc.tile_pool(name="pd3_big", bufs=1))
            w2_all = pd3_big.tile([128, HC, H2C, 128], BF16)   # 4 MB
            for oc in range(HC):
                nc.sync.dma_start(out=w2_all[:, oc], in_=d_w2[oc])
            yt_sb = pd3_big.tile([128, HC, R], F32R)           # 4 MB
            pd3_o = sd3.enter_context(tc.tile_pool(name="pd3_o", bufs=2))
            with tc.tile_pool(name="pd3_ps", bufs=2, space="PSUM") as pd3_ps:
                for oc in range(HC):
                    ps = pd3_ps.tile([128, R], F32, tag="w2ps", name="w2ps")
                    for rt in range(RT):
                        sl = bass.ts(rt, 512)
                        for hc in range(H2C):
                            nc.tensor.matmul(ps[:, sl], w2_all[:, oc, hc, :],
                                             h1_sb[:, hc, sl],
                                             start=(hc == 0),
                                             stop=(hc == H2C - 1))
                    ytmp = pd3_o.tile([128, R], F32, tag="ytmp", name="ytmp")
                    nc.vector.scalar_tensor_tensor(
                        out=ytmp, in0=ps, scalar=b2_sb[:, oc:oc + 1],
                        in1=sigb_sb[:, oc, :], op0=OP.add, op1=OP.mult)
                    nc.gpsimd.tensor_add(yt_sb[:, oc, :], ytmp,
                                         xt_sb[:, oc, :].bitcast(F32))

            # ====== Phase D4: final layernorm (gamma/beta folded) + out ====
            pd3_st = sd3.enter_context(
                tc.tile_pool(name="pd3_st", bufs=1, space="PSUM"))
            pd3_tp = sd3.enter_context(
                tc.tile_pool(name="pd3_tp", bufs=2, space="PSUM"))
            for rt in range(RT):
                sl = bass.ts(rt, 512)
                muy = pd3_st.tile([128, 512], F32, tag="muy", name="muy")
                msy = pd3_st.tile([128, 512], F32, tag="msy", name="msy")
                for oc in range(HC):
                    sqy = pd3_o.tile([128, 512], F32R, tag="sqy", name="sqy")
                    nc.vector.tensor_mul(sqy, yt_sb[:, oc, sl].bitcast(F32),
                                         yt_sb[:, oc, sl].bitcast(F32))
                    nc.tensor.matmul(muy, ones1k, yt_sb[:, oc, sl],
                                     start=(oc == 0), stop=(oc == HC - 1))
                    nc.tensor.matmul(msy, ones1k, sqy,
                                     start=(oc == 0), stop=(oc == HC - 1))
                muy_f = pd3_o.tile([128, 512], F32, tag="muyf", name="muyf",
                                   bufs=1)
                nc.scalar.activation(out=muy_f, in_=muy, func=AF.Copy)
                var = pd3_o.tile([128, 512], F32, tag="vary", name="vary",
                                 bufs=1)
                nc.vector.tensor_mul(var, muy_f, muy_f)
                nc.vector.tensor_sub(var, msy, var)
                nc.scalar.activation(out=var, in_=var, func=AF.Ln,
                                     bias=eps_t, scale=1.0)
                rstdy = pd3_o.tile([128, 512], F32, tag="rsty", name="rstdy",
                                   bufs=1)
                nc.scalar.activation(out=rstdy, in_=var, func=AF.Exp,
                                     scale=-0.5)
                # y_norm = (y - mu)*rstd*l2g + l2b, all feature-major
                for oc in range(HC):
                    t = pd3_o.tile([128, 512], F32, tag="ynt", name="ynt")
                    nc.vector.tensor_sub(t, yt_sb[:, oc, sl].bitcast(F32),
                                         muy_f)
                    nc.vector.scalar_tensor_tensor(
                        out=t, in0=t, scalar=l2g_sb[:, oc:oc + 1],
                        in1=rstdy, op0=OP.mult, op1=OP.mult)
                    nc.scalar.activation(out=yt_sb[:, oc, sl], in_=t,
                                         func=AF.Identity,
                                         bias=l2b_sb[:, oc:oc + 1])
                for j in range(4):
                    rc = rt * 4 + j
                    c0 = rt * 512 + j * 128
                    tp = pd3_tp.tile([128, H], F32R, tag="tp", name="tp")
                    for oc in range(HC):
                        nc.tensor.transpose(
                            tp[:, oc * 128:(oc + 1) * 128],
                            yt_sb[:, oc, c0:c0 + 128], ident)
                    yr = pd3_o.tile([128, H], F32, tag="yr", name="yr")
                    nc.scalar.activation(out=yr, in_=tp.bitcast(F32),
                                         func=AF.Copy)
                    nc.sync.dma_start(out=d_out[rc * 128:(rc + 1) * 128, :],
                                      in_=yr)

    nc.compile()
    return nc


_NC_CACHE = []


def _get_nc():
    if not _NC_CACHE:
        _NC_CACHE.append(build_program())
    return _NC_CACHE[0]


def _fp8(a, scale=1.0):
    return np.asarray(np.asarray(a, np.float32) * scale).astype(
        ml_dtypes.float8_e4m3)


def _chunk(w_t, dtype, scale=1.0):
    # [IN, OUT] -> [OUT//128, 128, IN//128, 128] contiguous per-partition
    inn, out = w_t.shape
    r = (w_t * scale).reshape(inn // 128, 128, out // 128, 128)
    r = r.transpose(2, 1, 0, 3)
    return np.ascontiguousarray(r.astype(dtype))


def kernel(query_hidden, mem_keys, importance, recency, access_count,
           Wq, bq, in_w, in_b, out_w, out_b, gate_w, gate_b,
           int_w1, int_b1, int_ln_g, int_ln_b, int_w2, int_b2,
           ln1_g, ln1_b, ln2_g, ln2_b, sel_params, top_k):
    np32 = lambda a: np.asarray(a, dtype=np.float32)
    query_hidden = np32(query_hidden)
    mem_keys = np32(mem_keys)
    top_k = int(top_k)
    assert top_k == K, f"kernel compiled for top_k={K}, got {top_k}"

    # HTPS selection (host): softmax-weighted score, top-k set, gather.
    sp = np32(sel_params)
    w = np.exp(sp - sp.max())
    w = w / w.sum()
    acc = np32(access_count)
    sel = w[0] * np32(importance) + w[1] * np32(recency) + w[2] * (acc / acc.max())
    idx = np.argpartition(-sel, top_k - 1)[:top_k]
    mem = mem_keys[idx]                                  # [K, H]

    # memory layernorm on host (ln1 gamma/beta folded into wk/wv below)
    mu = mem.mean(1, keepdims=True)
    var = ((mem - mu) ** 2).mean(1, keepdims=True)
    mem_n = (mem - mu) / np.sqrt(var + EPS)
    mn8_t = np.ascontiguousarray(_fp8(mem_n.T))          # [H, K] fp8

    in_w = np32(in_w)
    in_b = np32(in_b)
    wq, wk, wv = in_w[:H], in_w[H:2 * H], in_w[2 * H:]
    bqi, bki, bvi = in_b[:H], in_b[H:2 * H], in_b[2 * H:]
    wc = wq @ np32(Wq)                                   # fused query proj
    bc = wq @ np32(bq) + bqi

    g1 = np32(ln1_g)
    b1v = np32(ln1_b)
    bki = bki + wk @ b1v
    bvi = bvi + wv @ b1v
    wk = wk * g1[None, :]
    wv = wv * g1[None, :]

    out_w = np32(out_w)
    out_b = np32(out_b)
    gate_w = np32(gate_w)
    int_w1 = np32(int_w1)
    gwx, gwa = gate_w[:, :H], gate_w[:, H:]
    w1x, w1a = int_w1[:, :H], int_w1[:, H:]
    gate_b_f = np32(gate_b) + gwa @ out_b
    int_b1_f = np32(int_b1) + w1a @ out_b

    T = lambda a: np.ascontiguousarray(np32(a).T)
    gw_t = np.concatenate([gwx.T, (gwa @ out_w).T], axis=0)   # [2H, H]
    w1_t = np.concatenate([w1x.T, (w1a @ out_w).T], axis=0)   # [2H, 2H]

    # wv8: [128, HC, H] = wv.T reshaped (in-chunk-major partitions)
    wv_t = (T(wv) * WS).reshape(HC, 128, H).transpose(1, 0, 2)
    common = {
        "mn8_t": mn8_t,
        "wc8": _chunk(T(wc), ml_dtypes.float8_e4m3, WS),
        "wk8": _chunk(T(wk), ml_dtypes.float8_e4m3, WS),
        "wv8": np.ascontiguousarray(wv_t.astype(ml_dtypes.float8_e4m3)),
        "gw8": _chunk(gw_t, ml_dtypes.float8_e4m3, WS),
        "w1b": _chunk(w1_t, ml_dtypes.float8_e4m3, WS),
        "w2b": _chunk(T(np32(int_w2)), ml_dtypes.bfloat16),
        "bc8": QS * bc, "bk8": QS * bki, "bv": bvi,
        "gate_b": gate_b_f, "int_b1": int_b1_f, "int_b2": np32(int_b2),
        "iln_g": np32(int_ln_g), "iln_b": np32(int_ln_b),
        "ln2_g": np32(ln2_g), "ln2_b": np32(ln2_b),
    }
    X = query_hidden.reshape(B * S, H)
    in_maps = []
    for c in range(N_CORES):
        m = dict(common)
        xc_t = np.ascontiguousarray(X[c * R:(c + 1) * R].T)
        m["x_t"] = xc_t
        m["x8_t"] = np.ascontiguousarray(_fp8(xc_t))
        in_maps.append(m)

    nc = _get_nc()
    res = run_bass_kernel_spmd(nc, in_maps, core_ids=list(range(N_CORES)))
    out = np.empty((B * S, H), dtype=np.float32)
    for c in range(N_CORES):
        out[c * R:(c + 1) * R] = res.results[c]["out"]
    return out.reshape(B, S, H)


# revision 18
# speedup vs baseline: 1.8660x; 1.2632x over previous
"""MemoryRetriever kernel for 8x Trainium2 NeuronCores.

Data-parallel over the B*S=8192 query rows (1024 rows/core); the selected
memory bank and all weights are replicated.

Precision plan (validated vs reference on CPU and HW):
  - K/V/Q projections, attention scores/softmax/context, the sigmoid gate,
    and h1 = cat @ w1 run in fp8e4 (e4m3) with DoubleRow matmuls (two
    128-row contraction chunks per instruction, 2x fp32r throughput on HW).
    fp8 weights are prescaled x64 on host; Q/K activations stored x8.
  - integ = gelu(h1n) @ w2 runs in bf16 (fp8 there exceeds tolerance).
  Expected rel err ~1.5e-2 < 2e-2 gate (CPU sim of this exact config).

Host-side fusions (exact up to rounding): fused query projection
(wq_in@Wq), memory layernorm (device gets mem_n as fp8 directly), ln1
gamma/beta folded into wk/wv, attn_out = ctx@out_w.T + out_b folded into
the gate/integration weights so cat == [x; ctx] on device.

Device layout is feature-major: [feature chunk of 128 partitions, rows].
K and V stay SBUF-resident (fp8, 2 MB each) - no DRAM roundtrip.
The final layernorm folds ln2 gamma/beta into the feature-major
normalize (per-partition scale/bias), then PE-transposes to row-major.
"""

import sys
from contextlib import ExitStack

if "/opt/trn_rl_repo" not in sys.path:
    sys.path.insert(0, "/opt/trn_rl_repo")

import numpy as np
import ml_dtypes

import concourse.bass as bass
import concourse.mybir as mybir
import concourse.tile as tile
from concourse import bacc
from concourse.bass_utils import run_bass_kernel_spmd
from concourse.masks import make_identity

F32 = mybir.dt.float32
F32R = mybir.dt.float32r
BF16 = mybir.dt.bfloat16
F8 = mybir.dt.float8e4
AF = mybir.ActivationFunctionType
OP = mybir.AluOpType
DR = mybir.MatmulPerfMode.DoubleRow

H = 1024
NH = 4
HD = H // NH          # 256
K = 2048              # top_k
B, S = 4, 2048
N_CORES = 8
R = (B * S) // N_CORES  # 1024 rows per core
EPS = 1e-5
H2 = 2 * H            # 2048

HC = H // 128         # 8 feature chunks
H2C = H2 // 128       # 16
KC = K // 128         # 16 key chunks
RT = R // 512         # 2 row tiles of 512
KT4 = K // 512        # 4 key tiles of 512
WS = 64.0             # fp8 weight prescale
QS = 8.0              # fp8 Q/K activation prescale


def build_program():
    nc = bacc.Bacc("TRN2", target_bir_lowering=False)

    d_xt = nc.declare_dram_parameter("x_t", [H, R], F32R, isOutput=False)
    d_x8 = nc.declare_dram_parameter("x8_t", [H, R], F8, isOutput=False)
    d_mn8 = nc.declare_dram_parameter("mn8_t", [H, K], F8, isOutput=False)
    d_wc = nc.declare_dram_parameter("wc8", [HC, 128, HC, 128], F8,
                                     isOutput=False)
    d_wk = nc.declare_dram_parameter("wk8", [HC, 128, HC, 128], F8,
                                     isOutput=False)
    d_wv = nc.declare_dram_parameter("wv8", [128, HC, H], F8, isOutput=False)
    d_gw = nc.declare_dram_parameter("gw8", [HC, 128, H2C, 128], F8,
                                     isOutput=False)
    d_w1 = nc.declare_dram_parameter("w1b", [H2C, 128, H2C, 128], F8,
                                     isOutput=False)
    d_w2 = nc.declare_dram_parameter("w2b", [HC, 128, H2C, 128], BF16,
                                     isOutput=False)
    d_bc = nc.declare_dram_parameter("bc8", [H], F32, isOutput=False)
    d_bk = nc.declare_dram_parameter("bk8", [H], F32, isOutput=False)
    d_bv = nc.declare_dram_parameter("bv", [H], F32, isOutput=False)
    d_gb = nc.declare_dram_parameter("gate_b", [H], F32, isOutput=False)
    d_b1 = nc.declare_dram_parameter("int_b1", [H2], F32, isOutput=False)
    d_b2 = nc.declare_dram_parameter("int_b2", [H], F32, isOutput=False)
    d_ilg = nc.declare_dram_parameter("iln_g", [H2], F32, isOutput=False)
    d_ilb = nc.declare_dram_parameter("iln_b", [H2], F32, isOutput=False)
    d_l2g = nc.declare_dram_parameter("ln2_g", [H], F32, isOutput=False)
    d_l2b = nc.declare_dram_parameter("ln2_b", [H], F32, isOutput=False)
    d_out = nc.declare_dram_parameter("out", [R, H], F32, isOutput=True)

    with tile.TileContext(nc) as tc, ExitStack() as top:
        singles = top.enter_context(tc.tile_pool(name="singles", bufs=1))

        ident_f = singles.tile([128, 128], F32)
        make_identity(nc, ident_f)
        ident = singles.tile([128, 128], F32R)
        nc.scalar.activation(out=ident, in_=ident_f, func=AF.Copy)
        scratch1 = singles.tile([128, 128], F32)
        nc.vector.memset(scratch1, 1.0)
        ones8 = singles.tile([128, 2, 128], F8)
        nc.scalar.activation(out=ones8[:, 0], in_=scratch1, func=AF.Copy)
        nc.scalar.activation(out=ones8[:, 1], in_=scratch1, func=AF.Copy)
        ones1k = singles.tile([128, 128], F32R)
        nc.scalar.activation(out=ones1k, in_=scratch1, func=AF.Copy,
                             scale=1.0 / 1024.0)
        ones2kb = singles.tile([128, 128], BF16)
        nc.scalar.activation(out=ones2kb, in_=scratch1, func=AF.Copy,
                             scale=1.0 / 2048.0)
        eps_t = singles.tile([128, 1], F32)
        nc.vector.memset(eps_t, EPS)

        def load_pp(vec, n, nm):  # [n*128] dram vector -> [128, n]
            t = singles.tile([128, n], F32, tag=f"pp_{nm}", name=f"pp_{nm}")
            nc.sync.dma_start(out=t, in_=vec[:].rearrange("(c p) -> p c", p=128))
            return t

        bc8_sb = load_pp(d_bc, HC, "bc")
        bk8_sb = load_pp(d_bk, HC, "bk")
        bv_sb = load_pp(d_bv, HC, "bv")
        gb_sb = load_pp(d_gb, HC, "gb")
        b1_sb = load_pp(d_b1, H2C, "b1")
        b2_sb = load_pp(d_b2, HC, "b2")
        ilg_sb = load_pp(d_ilg, H2C, "ilg")
        ilb_sb = load_pp(d_ilb, H2C, "ilb")
        l2g_sb = load_pp(d_l2g, HC, "l2g")
        l2b_sb = load_pp(d_l2b, HC, "l2b")
        xt_sb = singles.tile([128, HC, R], F32R)      # 4 MB, residual

        # pool lifetimes (stack/LIFO): pDlong outlives pBC outlives pA
        pDlong = top.enter_context(tc.tile_pool(name="pDlong", bufs=1))
        pBC = tc.alloc_tile_pool(name="pBC", bufs=1)  # released after D1

        x8_sb = pBC.tile([128, HC, R], F8)            # 1 MB
        ksb = pBC.tile([128, HC, K], F8)              # 2 MB: K^T, 8*K_true
        vsb = pBC.tile([128, KC, H], F8)              # 2 MB: V rows
        q8_sb = pBC.tile([128, HC, R], F8)            # 1 MB: 8*Q
        ctx8_sb = pBC.tile([128, HC, R], F8)          # 1 MB: fp8 ctx (+bv)

        def cat8(c):   # fp8 cat chunk c in [0, 16)
            return x8_sb if c < HC else ctx8_sb

        # =========== Phase A: K/V projections (fp8 DoubleRow) ===========
        with ExitStack() as sa:
            pa_in = sa.enter_context(tc.tile_pool(name="pa_in", bufs=1))
            mn8_sb = pa_in.tile([128, HC, K], F8)     # 2 MB, layernormed mem
            nc.sync.dma_start(
                out=mn8_sb, in_=d_mn8[:].rearrange("(c p) k -> p c k", p=128))
            wv8_sb = pa_in.tile([128, HC, H], F8)     # 1 MB
            nc.sync.dma_start(out=wv8_sb, in_=d_wv[:])
            nc.sync.dma_start(
                out=x8_sb, in_=d_x8[:].rearrange("(c p) r -> p c r", p=128))
            pa_w = sa.enter_context(tc.tile_pool(name="pa_w", bufs=3))
            pa_ps = sa.enter_context(
                tc.tile_pool(name="pa_ps", bufs=2, space="PSUM"))
            for oc in range(HC):
                wks = pa_w.tile([128, HC, 128], F8, tag="wk", name="wks")
                nc.sync.dma_start(out=wks, in_=d_wk[oc])
                for kt in range(KT4):
                    sl = bass.ts(kt, 512)
                    ps = pa_ps.tile([128, 512], F32, tag="kps", name="kps")
                    for p in range(4):
                        nc.tensor.matmul(ps, wks[:, 2 * p:2 * p + 2, :],
                                         mn8_sb[:, 2 * p:2 * p + 2, sl],
                                         start=(p == 0), stop=(p == 3),
                                         perf_mode=DR)
                    nc.scalar.activation(out=ksb[:, oc, sl], in_=ps,
                                         func=AF.Identity, scale=1.0 / 8.0,
                                         bias=bk8_sb[:, oc:oc + 1])
            for kc in range(KC):
                ps = pa_ps.tile([128, H], F32, tag="vps", name="vps")
                for half in range(2):
                    hsl = bass.ts(half, 512)
                    for p in range(4):
                        nc.tensor.matmul(
                            ps[:, hsl],
                            mn8_sb[:, 2 * p:2 * p + 2, kc * 128:(kc + 1) * 128],
                            wv8_sb[:, 2 * p:2 * p + 2, hsl],
                            start=(p == 0), stop=(p == 3), perf_mode=DR)
                # V stored unscaled: psum = 64*V -> scale 1/64
                nc.vector.tensor_scalar_mul(vsb[:, kc, :], ps, 1.0 / WS)

        # =========== Phase B: query projection (fp8 DoubleRow) ===========
        with ExitStack() as sb_:
            pb_w = sb_.enter_context(tc.tile_pool(name="pb_w", bufs=3))
            pb_ps = sb_.enter_context(
                tc.tile_pool(name="pb_ps", bufs=2, space="PSUM"))
            for oc in range(HC):
                wcs = pb_w.tile([128, HC, 128], F8, tag="wc", name="wcs")
                nc.sync.dma_start(out=wcs, in_=d_wc[oc])
                ps = pb_ps.tile([128, R], F32, tag="qps", name="qps")
                for rt in range(RT):
                    sl = bass.ts(rt, 512)
                    for p in range(4):
                        nc.tensor.matmul(ps[:, sl],
                                         wcs[:, 2 * p:2 * p + 2, :],
                                         x8_sb[:, 2 * p:2 * p + 2, sl],
                                         start=(p == 0), stop=(p == 3),
                                         perf_mode=DR)
                nc.scalar.activation(out=q8_sb[:, oc, :], in_=ps,
                                     func=AF.Identity, scale=1.0 / 8.0,
                                     bias=bc8_sb[:, oc:oc + 1])

        # prefetch phase-D3 weights and the residual during phase C
        w2_all = pDlong.tile([128, HC, H2C, 128], BF16)    # 4 MB
        for oc in range(HC):
            nc.sync.dma_start(out=w2_all[:, oc], in_=d_w2[oc])
        nc.sync.dma_start(
            out=xt_sb, in_=d_xt[:].rearrange("(c p) r -> p c r", p=128))

        # =========== Phase C: attention (fp8 DoubleRow + exp pipeline) =====
        with ExitStack() as sc_:
            pc_e = sc_.enter_context(tc.tile_pool(name="pc_e", bufs=3))
            pc_o = sc_.enter_context(tc.tile_pool(name="pc_o", bufs=3))
            pc_sc = sc_.enter_context(
                tc.tile_pool(name="pc_sc", bufs=2, space="PSUM"))
            pc_acc = sc_.enter_context(
                tc.tile_pool(name="pc_acc", bufs=1, space="PSUM"))
            for h in range(NH):
                for qt in range(RT):
                    qsl = bass.ts(qt, 512)
                    sums = pc_acc.tile([128, 512], F32, tag="sums", name="sums")
                    ctx0 = pc_acc.tile([128, 512], F32, tag="ctx0", name="ctx0")
                    ctx1 = pc_acc.tile([128, 512], F32, tag="ctx1", name="ctx1")
                    e2s = [None] * (KC // 2)

                    def emit_scores_exp(pp):
                        sc = pc_sc.tile([128, 2, 512], F32, tag="sc", name="sc")
                        for j in range(2):
                            kp = 2 * pp + j
                            nc.tensor.matmul(
                                sc[:, j],
                                ksb[:, 2 * h:2 * h + 2,
                                    kp * 128:(kp + 1) * 128],
                                q8_sb[:, 2 * h:2 * h + 2, qsl],
                                start=True, stop=True, perf_mode=DR)
                        e2s[pp] = pc_e.tile([128, 2, 512], F8, tag="e2",
                                            name="e2")
                        nc.scalar.activation(out=e2s[pp], in_=sc, func=AF.Exp,
                                             scale=1.0 / 1024.0)

                    def emit_acc(pp):
                        e2 = e2s[pp]
                        st, sp = (pp == 0), (pp == KC // 2 - 1)
                        nc.tensor.matmul(sums, ones8, e2, start=st, stop=sp,
                                         perf_mode=DR)
                        nc.tensor.matmul(
                            ctx0, vsb[:, 2 * pp:2 * pp + 2,
                                      h * HD:h * HD + 128],
                            e2, start=st, stop=sp, perf_mode=DR)
                        nc.tensor.matmul(
                            ctx1, vsb[:, 2 * pp:2 * pp + 2,
                                      h * HD + 128:h * HD + 256],
                            e2, start=st, stop=sp, perf_mode=DR)

                    # PE order: sc(0) sc(1) acc(0) sc(2) acc(1) ... acc(7)
                    # so each exp has a pair of score-matmuls running behind it
                    for pp in range(KC // 2):
                        emit_scores_exp(pp)
                        if pp >= 1:
                            emit_acc(pp - 1)
                    emit_acc(KC // 2 - 1)

                    rec = pc_o.tile([128, 512], F32, tag="rec", name="rec")
                    nc.vector.reciprocal_approx_fast(out=rec, in_=sums)
                    for j, ctx in enumerate((ctx0, ctx1)):
                        t = pc_o.tile([128, 512], F32, tag="ctmp", name="ctmp")
                        nc.vector.tensor_mul(t, ctx, rec)
                        nc.vector.tensor_scalar_add(
                            ctx8_sb[:, 2 * h + j, qsl], t,
                            bv_sb[:, 2 * h + j:2 * h + j + 1])

        # =========== Phase D0: gate (fp8 DoubleRow) ===========
        sigb_sb = pDlong.tile([128, HC, R], BF16)     # 2 MB
        with ExitStack() as sd0:
            pd0_w = sd0.enter_context(tc.tile_pool(name="pd0_w", bufs=2))
            pd0_ps = sd0.enter_context(
                tc.tile_pool(name="pd0_ps", bufs=2, space="PSUM"))
            for oc in range(HC):
                gws = pd0_w.tile([128, H2C, 128], F8, tag="gw", name="gws")
                nc.sync.dma_start(out=gws, in_=d_gw[oc])
                ps = pd0_ps.tile([128, R], F32, tag="gps", name="gps")
                for rt in range(RT):
                    sl = bass.ts(rt, 512)
                    for p in range(8):
                        c = 2 * p
                        src = cat8(c)
                        cc = c if c < HC else c - HC
                        nc.tensor.matmul(ps[:, sl], gws[:, c:c + 2, :],
                                         src[:, cc:cc + 2, sl],
                                         start=(p == 0), stop=(p == 7),
                                         perf_mode=DR)
                nc.scalar.activation(out=sigb_sb[:, oc, :], in_=ps,
                                     func=AF.Sigmoid, scale=1.0 / WS,
                                     bias=gb_sb[:, oc:oc + 1])

        # ======= Phase D1: h1 = cat8 @ w1.T + b1 (fp8 DR), inline stats ====
        h1_sb = pDlong.tile([128, H2C, R], BF16)      # 4 MB
        mu2_f = pDlong.tile([128, R], F32)
        rstd2_f = pDlong.tile([128, R], F32)
        with ExitStack() as sd1:
            pd1_w = sd1.enter_context(tc.tile_pool(name="pd1_w", bufs=3))
            pd1_sq = sd1.enter_context(tc.tile_pool(name="pd1_sq", bufs=3))
            pd1_ps = sd1.enter_context(
                tc.tile_pool(name="pd1_ps", bufs=2, space="PSUM"))
            pd1_st = sd1.enter_context(
                tc.tile_pool(name="pd1_st", bufs=1, space="PSUM"))
            mu2_ps = [pd1_st.tile([128, 512], F32, tag=f"mu{i}", name=f"mu{i}")
                      for i in range(RT)]
            ms2_ps = [pd1_st.tile([128, 512], F32, tag=f"ms{i}", name=f"ms{i}")
                      for i in range(RT)]
            for oc2 in range(H2C):
                w1s = pd1_w.tile([128, H2C, 128], F8, tag="w1", name="w1s")
                nc.sync.dma_start(out=w1s, in_=d_w1[oc2])
                ps = pd1_ps.tile([128, R], F32, tag="h1ps", name="h1ps")
                for rt in range(RT):
                    sl = bass.ts(rt, 512)
                    for p in range(8):
                        c = 2 * p
                        src = cat8(c)
                        cc = c if c < HC else c - HC
                        nc.tensor.matmul(ps[:, sl], w1s[:, c:c + 2, :],
                                         src[:, cc:cc + 2, sl],
                                         start=(p == 0), stop=(p == 7),
                                         perf_mode=DR)
                nc.scalar.activation(out=h1_sb[:, oc2, :], in_=ps,
                                     func=AF.Identity, scale=1.0 / WS,
                                     bias=b1_sb[:, oc2:oc2 + 1])
                sq = pd1_sq.tile([128, R], BF16, tag="sq", name="sq")
                nc.vector.tensor_mul(sq, h1_sb[:, oc2, :], h1_sb[:, oc2, :])
                for rt in range(RT):
                    sl = bass.ts(rt, 512)
                    nc.tensor.matmul(mu2_ps[rt], ones2kb, h1_sb[:, oc2, sl],
                                     start=(oc2 == 0), stop=(oc2 == H2C - 1))
                    nc.tensor.matmul(ms2_ps[rt], ones2kb, sq[:, sl],
                                     start=(oc2 == 0), stop=(oc2 == H2C - 1))
            for rt in range(RT):
                sl = bass.ts(rt, 512)
                nc.scalar.activation(out=mu2_f[:, sl], in_=mu2_ps[rt],
                                     func=AF.Copy)
                var = pd1_sq.tile([128, 512], F32, tag="var", name="var")
                nc.vector.tensor_mul(var, mu2_f[:, sl], mu2_f[:, sl])
                nc.vector.tensor_sub(var, ms2_ps[rt], var)
                nc.scalar.activation(out=var, in_=var, func=AF.Ln,
                                     bias=eps_t, scale=1.0)
                nc.scalar.activation(out=rstd2_f[:, sl], in_=var,
                                     func=AF.Exp, scale=-0.5)

        pBC.release()   # frees ksb/vsb/q8/x8/ctx8 (7 MB)

        # =========== Phase D2: layernorm apply + gelu (in place) ===========
        with ExitStack() as sd2:
            pd2 = sd2.enter_context(tc.tile_pool(name="pd2", bufs=3))
            for oc2 in range(H2C):
                t1 = pd2.tile([128, R], F32, tag="t1", name="t1")
                if oc2 % 2 == 0:
                    nc.gpsimd.tensor_sub(t1, h1_sb[:, oc2, :], mu2_f)
                else:
                    nc.vector.tensor_sub(t1, h1_sb[:, oc2, :], mu2_f)
                nc.vector.scalar_tensor_tensor(
                    out=t1, in0=t1, scalar=ilg_sb[:, oc2:oc2 + 1],
                    in1=rstd2_f, op0=OP.mult, op1=OP.mult)
                nc.scalar.activation(out=h1_sb[:, oc2, :], in_=t1,
                                     func=AF.Gelu,
                                     bias=ilb_sb[:, oc2:oc2 + 1])

        # ====== Phase D3+D4: integ, y = x + gate*integ, final layernorm ====
        with ExitStack() as sd3:
            pd3_big = sd3.enter_context(tc.tile_pool(name="pd3_big", bufs=1))
            yt_sb = pd3_big.tile([128, HC, R], F32R)           # 4 MB
            pd3_o = sd3.enter_context(tc.tile_pool(name="pd3_o", bufs=2))
            pd3_ps = sd3.enter_context(
                tc.tile_pool(name="pd3_ps", bufs=2, space="PSUM"))
            pd3_st = sd3.enter_context(
                tc.tile_pool(name="pd3_st", bufs=1, space="PSUM"))
            pd3_tp = sd3.enter_context(
                tc.tile_pool(name="pd3_tp", bufs=2, space="PSUM"))
            for rt in range(RT):
                sl = bass.ts(rt, 512)
                for oc in range(HC):
                    ps = pd3_ps.tile([128, 512], F32, tag="w2ps", name="w2ps")
                    for hc in range(H2C):
                        nc.tensor.matmul(ps, w2_all[:, oc, hc, :],
                                         h1_sb[:, hc, sl],
                                         start=(hc == 0), stop=(hc == H2C - 1))
                    ytmp = pd3_o.tile([128, 512], F32, tag="ytmp", name="ytmp")
                    nc.vector.scalar_tensor_tensor(
                        out=ytmp, in0=ps, scalar=b2_sb[:, oc:oc + 1],
                        in1=sigb_sb[:, oc, sl], op0=OP.add, op1=OP.mult)
                    nc.gpsimd.tensor_add(yt_sb[:, oc, sl], ytmp,
                                         xt_sb[:, oc, sl].bitcast(F32))
                # final layernorm for this row tile (gamma/beta folded in)
                muy = pd3_st.tile([128, 512], F32, tag="muy", name="muy")
                msy = pd3_st.tile([128, 512], F32, tag="msy", name="msy")
                for oc in range(HC):
                    sqy = pd3_o.tile([128, 512], F32R, tag="sqy", name="sqy")
                    nc.vector.tensor_mul(sqy, yt_sb[:, oc, sl].bitcast(F32),
                                         yt_sb[:, oc, sl].bitcast(F32))
                    nc.tensor.matmul(muy, ones1k, yt_sb[:, oc, sl],
                                     start=(oc == 0), stop=(oc == HC - 1))
                    nc.tensor.matmul(msy, ones1k, sqy,
                                     start=(oc == 0), stop=(oc == HC - 1))
                muy_f = pd3_o.tile([128, 512], F32, tag="muyf", name="muyf",
                                   bufs=1)
                nc.scalar.activation(out=muy_f, in_=muy, func=AF.Copy)
                var = pd3_o.tile([128, 512], F32, tag="vary", name="vary",
                                 bufs=1)
                nc.vector.tensor_mul(var, muy_f, muy_f)
                nc.vector.tensor_sub(var, msy, var)
                nc.scalar.activation(out=var, in_=var, func=AF.Ln,
                                     bias=eps_t, scale=1.0)
                rstdy = pd3_o.tile([128, 512], F32, tag="rsty", name="rstdy",
                                   bufs=1)
                nc.scalar.activation(out=rstdy, in_=var, func=AF.Exp,
                                     scale=-0.5)
                # y_norm = (y - mu)*rstd*l2g + l2b, all feature-major
                for oc in range(HC):
                    t = pd3_o.tile([128, 512], F32, tag="ynt", name="ynt")
                    nc.vector.tensor_sub(t, yt_sb[:, oc, sl].bitcast(F32),
                                         muy_f)
                    nc.vector.scalar_tensor_tensor(
                        out=t, in0=t, scalar=l2g_sb[:, oc:oc + 1],
                        in1=rstdy, op0=OP.mult, op1=OP.mult)
                    nc.scalar.activation(out=yt_sb[:, oc, sl], in_=t,
                                         func=AF.Identity,
                                         bias=l2b_sb[:, oc:oc + 1])
                for j in range(4):
                    rc = rt * 4 + j
                    c0 = rt * 512 + j * 128
                    tp = pd3_tp.tile([128, H], F32R, tag="tp", name="tp")
                    for oc in range(HC):
                        nc.tensor.transpose(
                            tp[:, oc * 128:(oc + 1) * 128],
                            yt_sb[:, oc, c0:c0 + 128], ident)
                    yr = pd3_o.tile([128, H], F32, tag="yr", name="yr")
                    nc.scalar.activation(out=yr, in_=tp.bitcast(F32),
                                         func=AF.Copy)
                    nc.sync.dma_start(out=d_out[rc * 128:(rc + 1) * 128, :],
                                      in_=yr)

    nc.compile()
    return nc


_NC_CACHE = []


def _get_nc():
    if not _NC_CACHE:
        _NC_CACHE.append(build_program())
    return _NC_CACHE[0]


def _fp8(a, scale=1.0):
    return np.asarray(np.asarray(a, np.float32) * scale).astype(
        ml_dtypes.float8_e4m3)


def _chunk(w_t, dtype, scale=1.0):
    # [IN, OUT] -> [OUT//128, 128, IN//128, 128] contiguous per-partition
    inn, out = w_t.shape
    r = (w_t * scale).reshape(inn // 128, 128, out // 128, 128)
    r = r.transpose(2, 1, 0, 3)
    return np.ascontiguousarray(r.astype(dtype))


def kernel(query_hidden, mem_keys, importance, recency, access_count,
           Wq, bq, in_w, in_b, out_w, out_b, gate_w, gate_b,
           int_w1, int_b1, int_ln_g, int_ln_b, int_w2, int_b2,
           ln1_g, ln1_b, ln2_g, ln2_b, sel_params, top_k):
    np32 = lambda a: np.asarray(a, dtype=np.float32)
    query_hidden = np32(query_hidden)
    mem_keys = np32(mem_keys)
    top_k = int(top_k)
    assert top_k == K, f"kernel compiled for top_k={K}, got {top_k}"

    # HTPS selection (host): softmax-weighted score, top-k set, gather.
    sp = np32(sel_params)
    w = np.exp(sp - sp.max())
    w = w / w.sum()
    acc = np32(access_count)
    sel = w[0] * np32(importance) + w[1] * np32(recency) + w[2] * (acc / acc.max())
    idx = np.argpartition(-sel, top_k - 1)[:top_k]
    mem = mem_keys[idx]                                  # [K, H]

    # memory layernorm on host (ln1 gamma/beta folded into wk/wv below)
    mu = mem.mean(1, keepdims=True)
    var = ((mem - mu) ** 2).mean(1, keepdims=True)
    mem_n = (mem - mu) / np.sqrt(var + EPS)
    mn8_t = np.ascontiguousarray(_fp8(mem_n.T))          # [H, K] fp8

    in_w = np32(in_w)
    in_b = np32(in_b)
    wq, wk, wv = in_w[:H], in_w[H:2 * H], in_w[2 * H:]
    bqi, bki, bvi = in_b[:H], in_b[H:2 * H], in_b[2 * H:]
    wc = wq @ np32(Wq)                                   # fused query proj
    bc = wq @ np32(bq) + bqi

    g1 = np32(ln1_g)
    b1v = np32(ln1_b)
    bki = bki + wk @ b1v
    bvi = bvi + wv @ b1v
    wk = wk * g1[None, :]
    wv = wv * g1[None, :]

    out_w = np32(out_w)
    out_b = np32(out_b)
    gate_w = np32(gate_w)
    int_w1 = np32(int_w1)
    gwx, gwa = gate_w[:, :H], gate_w[:, H:]
    w1x, w1a = int_w1[:, :H], int_w1[:, H:]
    gate_b_f = np32(gate_b) + gwa @ out_b
    int_b1_f = np32(int_b1) + w1a @ out_b

    T = lambda a: np.ascontiguousarray(np32(a).T)
    gw_t = np.concatenate([gwx.T, (gwa @ out_w).T], axis=0)   # [2H, H]
    w1_t = np.concatenate([w1x.T, (w1a @ out_w).T], axis=0)   # [2H, 2H]

    # wv8: [128, HC, H] = wv.T reshaped (in-chunk-major partitions)
    wv_t = (T(wv) * WS).reshape(HC, 128, H).transpose(1, 0, 2)
    common = {
        "mn8_t": mn8_t,
        "wc8": _chunk(T(wc), ml_dtypes.float8_e4m3, WS),
        "wk8": _chunk(T(wk), ml_dtypes.float8_e4m3, WS),
        "wv8": np.ascontiguousarray(wv_t.astype(ml_dtypes.float8_e4m3)),
        "gw8": _chunk(gw_t, ml_dtypes.float8_e4m3, WS),
        "w1b": _chunk(w1_t, ml_dtypes.float8_e4m3, WS),
        "w2b": _chunk(T(np32(int_w2)), ml_dtypes.bfloat16),
        "bc8": QS * bc, "bk8": QS * bki, "bv": bvi,
        "gate_b": gate_b_f, "int_b1": int_b1_f, "int_b2": np32(int_b2),
        "iln_g": np32(int_ln_g), "iln_b": np32(int_ln_b),
        "ln2_g": np32(ln2_g), "ln2_b": np32(ln2_b),
    }
    X = query_hidden.reshape(B * S, H)
    in_maps = []
    for c in range(N_CORES):
        m = dict(common)
        xc_t = np.ascontiguousarray(X[c * R:(c + 1) * R].T)
        m["x_t"] = xc_t
        m["x8_t"] = np.ascontiguousarray(_fp8(xc_t))
        in_maps.append(m)

    nc = _get_nc()
    res = run_bass_kernel_spmd(nc, in_maps, core_ids=list(range(N_CORES)))
    out = np.empty((B * S, H), dtype=np.float32)
    for c in range(N_CORES):
        out[c * R:(c + 1) * R] = res.results[c]["out"]
    return out.reshape(B, S, H)


# revision 20
# speedup vs baseline: 1.8919x; 1.0139x over previous
"""MemoryRetriever kernel for 8x Trainium2 NeuronCores.

Data-parallel over the B*S=8192 query rows (1024 rows/core); the selected
memory bank and all weights are replicated.

Precision plan (validated vs reference on CPU and HW):
  - K/V/Q projections, attention scores/softmax/context, the sigmoid gate,
    and h1 = cat @ w1 run in fp8e4 (e4m3) with DoubleRow matmuls (two
    128-row contraction chunks per instruction, 2x fp32r throughput on HW).
    fp8 weights are prescaled x64 on host; Q/K activations stored x8.
  - integ = gelu(h1n) @ w2 runs in bf16 (fp8 there exceeds tolerance).
  Expected rel err ~1.5e-2 < 2e-2 gate (CPU sim of this exact config).

Host-side fusions (exact up to rounding): fused query projection
(wq_in@Wq), memory layernorm (device gets mem_n as fp8 directly), ln1
gamma/beta folded into wk/wv, attn_out = ctx@out_w.T + out_b folded into
the gate/integration weights so cat == [x; ctx] on device.

Device layout is feature-major: [feature chunk of 128 partitions, rows].
K and V stay SBUF-resident (fp8, 2 MB each) - no DRAM roundtrip.
The final layernorm folds ln2 gamma/beta into the feature-major
normalize (per-partition scale/bias), then PE-transposes to row-major.
"""

import sys
from contextlib import ExitStack

if "/opt/trn_rl_repo" not in sys.path:
    sys.path.insert(0, "/opt/trn_rl_repo")

import numpy as np
import ml_dtypes

import concourse.bass as bass
import concourse.mybir as mybir
import concourse.tile as tile
from concourse import bacc
from concourse.bass_utils import run_bass_kernel_spmd
from concourse.masks import make_identity

F32 = mybir.dt.float32
F32R = mybir.dt.float32r
BF16 = mybir.dt.bfloat16
F8 = mybir.dt.float8e4
AF = mybir.ActivationFunctionType
OP = mybir.AluOpType
DR = mybir.MatmulPerfMode.DoubleRow

H = 1024
NH = 4
HD = H // NH          # 256
K = 2048              # top_k
B, S = 4, 2048
N_CORES = 8
R = (B * S) // N_CORES  # 1024 rows per core
EPS = 1e-5
H2 = 2 * H            # 2048

HC = H // 128         # 8 feature chunks
H2C = H2 // 128       # 16
KC = K // 128         # 16 key chunks
RT = R // 512         # 2 row tiles of 512
KT4 = K // 512        # 4 key tiles of 512
WS = 64.0             # fp8 weight prescale
QS = 8.0              # fp8 Q/K activation prescale


def build_program():
    nc = bacc.Bacc("TRN2", target_bir_lowering=False)

    d_xt = nc.declare_dram_parameter("x_t", [H, R], F32R, isOutput=False)
    d_x8 = nc.declare_dram_parameter("x8_t", [H, R], F8, isOutput=False)
    d_mn8 = nc.declare_dram_parameter("mn8_t", [H, K], F8, isOutput=False)
    d_wc = nc.declare_dram_parameter("wc8", [HC, 128, HC, 128], F8,
                                     isOutput=False)
    d_wk = nc.declare_dram_parameter("wk8", [HC, 128, HC, 128], F8,
                                     isOutput=False)
    d_wv = nc.declare_dram_parameter("wv8", [128, HC, H], F8, isOutput=False)
    d_gw = nc.declare_dram_parameter("gw8", [HC, 128, H2C, 128], F8,
                                     isOutput=False)
    d_w1 = nc.declare_dram_parameter("w1b", [H2C, 128, H2C, 128], F8,
                                     isOutput=False)
    d_w2 = nc.declare_dram_parameter("w2b", [HC, 128, H2C, 128], BF16,
                                     isOutput=False)
    # all bias-ish vectors concatenated: [bc8|bk8|bv|gate_b|int_b2|ln2_g|
    # ln2_b (H each) | int_b1|iln_g|iln_b (2H each)] = 13H total
    d_bias = nc.declare_dram_parameter("biases", [13 * H], F32, isOutput=False)
    d_out = nc.declare_dram_parameter("out", [R, H], F32, isOutput=True)

    with tile.TileContext(nc) as tc, ExitStack() as top:
        singles = top.enter_context(tc.tile_pool(name="singles", bufs=1))

        ident_f = singles.tile([128, 128], F32)
        make_identity(nc, ident_f)
        ident = singles.tile([128, 128], F32R)
        nc.scalar.activation(out=ident, in_=ident_f, func=AF.Copy)
        scratch1 = singles.tile([128, 128], F32)
        nc.vector.memset(scratch1, 1.0)
        ones8 = singles.tile([128, 2, 128], F8)
        nc.scalar.activation(out=ones8[:, 0], in_=scratch1, func=AF.Copy)
        nc.scalar.activation(out=ones8[:, 1], in_=scratch1, func=AF.Copy)
        ones1k = singles.tile([128, 128], F32R)
        nc.scalar.activation(out=ones1k, in_=scratch1, func=AF.Copy,
                             scale=1.0 / 1024.0)
        ones2kb = singles.tile([128, 128], BF16)
        nc.scalar.activation(out=ones2kb, in_=scratch1, func=AF.Copy,
                             scale=1.0 / 2048.0)
        eps_t = singles.tile([128, 1], F32)
        nc.vector.memset(eps_t, EPS)

        bias_sb = singles.tile([128, 13 * HC], F32, tag="biases",
                               name="bias_sb")
        nc.sync.dma_start(
            out=bias_sb, in_=d_bias[:].rearrange("(c p) -> p c", p=128))
        bc8_sb = bias_sb[:, 0:8]
        bk8_sb = bias_sb[:, 8:16]
        bv_sb = bias_sb[:, 16:24]
        gb_sb = bias_sb[:, 24:32]
        b2_sb = bias_sb[:, 32:40]
        l2g_sb = bias_sb[:, 40:48]
        l2b_sb = bias_sb[:, 48:56]
        b1_sb = bias_sb[:, 56:72]
        ilg_sb = bias_sb[:, 72:88]
        ilb_sb = bias_sb[:, 88:104]
        xt_sb = singles.tile([128, HC, R], F32R)      # 4 MB, residual

        # pool lifetimes (stack/LIFO): pDlong outlives pBC outlives pA
        pDlong = top.enter_context(tc.tile_pool(name="pDlong", bufs=1))
        pBC = tc.alloc_tile_pool(name="pBC", bufs=1)  # released after D1

        x8_sb = pBC.tile([128, HC, R], F8)            # 1 MB
        ksb = pBC.tile([128, HC, K], F8)              # 2 MB: K^T, 8*K_true
        vsb = pBC.tile([128, KC, H], F8)              # 2 MB: V rows
        q8_sb = pBC.tile([128, HC, R], F8)            # 1 MB: 8*Q
        ctx8_sb = pBC.tile([128, HC, R], F8)          # 1 MB: fp8 ctx (+bv)

        def cat8(c):   # fp8 cat chunk c in [0, 16)
            return x8_sb if c < HC else ctx8_sb

        # =========== Phase A: K/V projections (fp8 DoubleRow) ===========
        with ExitStack() as sa:
            pa_in = sa.enter_context(tc.tile_pool(name="pa_in", bufs=1))
            mn8_sb = pa_in.tile([128, HC, K], F8)     # 2 MB, layernormed mem
            nc.sync.dma_start(
                out=mn8_sb, in_=d_mn8[:].rearrange("(c p) k -> p c k", p=128))
            wv8_sb = pa_in.tile([128, HC, H], F8)     # 1 MB
            nc.sync.dma_start(out=wv8_sb, in_=d_wv[:])
            nc.sync.dma_start(
                out=x8_sb, in_=d_x8[:].rearrange("(c p) r -> p c r", p=128))
            pa_w = sa.enter_context(tc.tile_pool(name="pa_w", bufs=3))
            pa_ps = sa.enter_context(
                tc.tile_pool(name="pa_ps", bufs=2, space="PSUM"))
            for oc in range(HC):
                wks = pa_w.tile([128, HC, 128], F8, tag="wk", name="wks")
                nc.sync.dma_start(out=wks, in_=d_wk[oc])
                for kt in range(KT4):
                    sl = bass.ts(kt, 512)
                    ps = pa_ps.tile([128, 512], F32, tag="kps", name="kps")
                    for p in range(4):
                        nc.tensor.matmul(ps, wks[:, 2 * p:2 * p + 2, :],
                                         mn8_sb[:, 2 * p:2 * p + 2, sl],
                                         start=(p == 0), stop=(p == 3),
                                         perf_mode=DR)
                    nc.scalar.activation(out=ksb[:, oc, sl], in_=ps,
                                         func=AF.Identity, scale=1.0 / 8.0,
                                         bias=bk8_sb[:, oc:oc + 1])
            for kc in range(KC):
                ps = pa_ps.tile([128, H], F32, tag="vps", name="vps")
                for half in range(2):
                    hsl = bass.ts(half, 512)
                    for p in range(4):
                        nc.tensor.matmul(
                            ps[:, hsl],
                            mn8_sb[:, 2 * p:2 * p + 2, kc * 128:(kc + 1) * 128],
                            wv8_sb[:, 2 * p:2 * p + 2, hsl],
                            start=(p == 0), stop=(p == 3), perf_mode=DR)
                # V stored unscaled: psum = 64*V -> scale 1/64
                nc.vector.tensor_scalar_mul(vsb[:, kc, :], ps, 1.0 / WS)

        # =========== Phase B: query projection (fp8 DoubleRow) ===========
        with ExitStack() as sb_:
            pb_w = sb_.enter_context(tc.tile_pool(name="pb_w", bufs=3))
            pb_ps = sb_.enter_context(
                tc.tile_pool(name="pb_ps", bufs=2, space="PSUM"))
            for oc in range(HC):
                wcs = pb_w.tile([128, HC, 128], F8, tag="wc", name="wcs")
                nc.sync.dma_start(out=wcs, in_=d_wc[oc])
                ps = pb_ps.tile([128, R], F32, tag="qps", name="qps")
                for rt in range(RT):
                    sl = bass.ts(rt, 512)
                    for p in range(4):
                        nc.tensor.matmul(ps[:, sl],
                                         wcs[:, 2 * p:2 * p + 2, :],
                                         x8_sb[:, 2 * p:2 * p + 2, sl],
                                         start=(p == 0), stop=(p == 3),
                                         perf_mode=DR)
                nc.scalar.activation(out=q8_sb[:, oc, :], in_=ps,
                                     func=AF.Identity, scale=1.0 / 8.0,
                                     bias=bc8_sb[:, oc:oc + 1])

        # prefetch phase-D3 weights and the residual during phase C
        w2_all = pDlong.tile([128, HC, H2C, 128], BF16)    # 4 MB
        for oc in range(HC):
            nc.sync.dma_start(out=w2_all[:, oc], in_=d_w2[oc])
        nc.sync.dma_start(
            out=xt_sb, in_=d_xt[:].rearrange("(c p) r -> p c r", p=128))

        # =========== Phase C: attention (fp8 DoubleRow + exp pipeline) =====
        with ExitStack() as sc_:
            pc_e = sc_.enter_context(tc.tile_pool(name="pc_e", bufs=3))
            pc_o = sc_.enter_context(tc.tile_pool(name="pc_o", bufs=3))
            pc_sc = sc_.enter_context(
                tc.tile_pool(name="pc_sc", bufs=2, space="PSUM"))
            pc_acc = sc_.enter_context(
                tc.tile_pool(name="pc_acc", bufs=1, space="PSUM"))
            for h in range(NH):
                for qt in range(RT):
                    qsl = bass.ts(qt, 512)
                    sums = pc_acc.tile([128, 512], F32, tag="sums", name="sums")
                    ctx0 = pc_acc.tile([128, 512], F32, tag="ctx0", name="ctx0")
                    ctx1 = pc_acc.tile([128, 512], F32, tag="ctx1", name="ctx1")
                    e2s = [None] * (KC // 2)

                    def emit_scores_exp(pp):
                        sc = pc_sc.tile([128, 2, 512], F32, tag="sc", name="sc")
                        for j in range(2):
                            kp = 2 * pp + j
                            nc.tensor.matmul(
                                sc[:, j],
                                ksb[:, 2 * h:2 * h + 2,
                                    kp * 128:(kp + 1) * 128],
                                q8_sb[:, 2 * h:2 * h + 2, qsl],
                                start=True, stop=True, perf_mode=DR)
                        e2s[pp] = pc_e.tile([128, 2, 512], F8, tag="e2",
                                            name="e2")
                        nc.scalar.activation(out=e2s[pp], in_=sc, func=AF.Exp,
                                             scale=1.0 / 1024.0)

                    def emit_acc(pp):
                        e2 = e2s[pp]
                        st, sp = (pp == 0), (pp == KC // 2 - 1)
                        nc.tensor.matmul(sums, ones8, e2, start=st, stop=sp,
                                         perf_mode=DR)
                        nc.tensor.matmul(
                            ctx0, vsb[:, 2 * pp:2 * pp + 2,
                                      h * HD:h * HD + 128],
                            e2, start=st, stop=sp, perf_mode=DR)
                        nc.tensor.matmul(
                            ctx1, vsb[:, 2 * pp:2 * pp + 2,
                                      h * HD + 128:h * HD + 256],
                            e2, start=st, stop=sp, perf_mode=DR)

                    # PE order: sc(0) sc(1) acc(0) sc(2) acc(1) ... acc(7)
                    # so each exp has a pair of score-matmuls running behind it
                    for pp in range(KC // 2):
                        emit_scores_exp(pp)
                        if pp >= 1:
                            emit_acc(pp - 1)
                    emit_acc(KC // 2 - 1)

                    rec = pc_o.tile([128, 512], F32, tag="rec", name="rec")
                    nc.vector.reciprocal_approx_fast(out=rec, in_=sums)
                    for j, ctx in enumerate((ctx0, ctx1)):
                        t = pc_o.tile([128, 512], F32, tag="ctmp", name="ctmp")
                        nc.vector.tensor_mul(t, ctx, rec)
                        nc.vector.tensor_scalar_add(
                            ctx8_sb[:, 2 * h + j, qsl], t,
                            bv_sb[:, 2 * h + j:2 * h + j + 1])

        # ======= Phase D1: h1 = cat8 @ w1.T + b1 (fp8 DR), inline stats ====
        h1_sb = pDlong.tile([128, H2C, R], BF16)      # 4 MB
        mu2_f = pDlong.tile([128, R], F32)
        rstd2_f = pDlong.tile([128, R], F32)
        with ExitStack() as sd1:
            pd1_w = sd1.enter_context(tc.tile_pool(name="pd1_w", bufs=3))
            pd1_sq = sd1.enter_context(tc.tile_pool(name="pd1_sq", bufs=3))
            pd1_ps = sd1.enter_context(
                tc.tile_pool(name="pd1_ps", bufs=2, space="PSUM"))
            pd1_st = sd1.enter_context(
                tc.tile_pool(name="pd1_st", bufs=1, space="PSUM"))
            mu2_ps = [pd1_st.tile([128, 512], F32, tag=f"mu{i}", name=f"mu{i}")
                      for i in range(RT)]
            ms2_ps = [pd1_st.tile([128, 512], F32, tag=f"ms{i}", name=f"ms{i}")
                      for i in range(RT)]
            for oc2 in range(H2C):
                w1s = pd1_w.tile([128, H2C, 128], F8, tag="w1", name="w1s")
                nc.sync.dma_start(out=w1s, in_=d_w1[oc2])
                ps = pd1_ps.tile([128, R], F32, tag="h1ps", name="h1ps")
                for rt in range(RT):
                    sl = bass.ts(rt, 512)
                    for p in range(8):
                        c = 2 * p
                        src = cat8(c)
                        cc = c if c < HC else c - HC
                        nc.tensor.matmul(ps[:, sl], w1s[:, c:c + 2, :],
                                         src[:, cc:cc + 2, sl],
                                         start=(p == 0), stop=(p == 7),
                                         perf_mode=DR)
                nc.scalar.activation(out=h1_sb[:, oc2, :], in_=ps,
                                     func=AF.Identity, scale=1.0 / WS,
                                     bias=b1_sb[:, oc2:oc2 + 1])
                sq = pd1_sq.tile([128, R], BF16, tag="sq", name="sq")
                nc.vector.tensor_mul(sq, h1_sb[:, oc2, :], h1_sb[:, oc2, :])
                for rt in range(RT):
                    sl = bass.ts(rt, 512)
                    nc.tensor.matmul(mu2_ps[rt], ones2kb, h1_sb[:, oc2, sl],
                                     start=(oc2 == 0), stop=(oc2 == H2C - 1))
                    nc.tensor.matmul(ms2_ps[rt], ones2kb, sq[:, sl],
                                     start=(oc2 == 0), stop=(oc2 == H2C - 1))
            for rt in range(RT):
                sl = bass.ts(rt, 512)
                nc.scalar.activation(out=mu2_f[:, sl], in_=mu2_ps[rt],
                                     func=AF.Copy)
                var = pd1_sq.tile([128, 512], F32, tag="var", name="var")
                nc.vector.tensor_mul(var, mu2_f[:, sl], mu2_f[:, sl])
                nc.vector.tensor_sub(var, ms2_ps[rt], var)
                nc.scalar.activation(out=var, in_=var, func=AF.Ln,
                                     bias=eps_t, scale=1.0)
                nc.scalar.activation(out=rstd2_f[:, sl], in_=var,
                                     func=AF.Exp, scale=-0.5)

        # =========== Phase D0: gate (fp8 DoubleRow) ===========
        sigb_sb = pDlong.tile([128, HC, R], BF16)     # 2 MB
        with ExitStack() as sd0:
            pd0_w = sd0.enter_context(tc.tile_pool(name="pd0_w", bufs=2))
            pd0_ps = sd0.enter_context(
                tc.tile_pool(name="pd0_ps", bufs=2, space="PSUM"))
            for oc in range(HC):
                gws = pd0_w.tile([128, H2C, 128], F8, tag="gw", name="gws")
                nc.sync.dma_start(out=gws, in_=d_gw[oc])
                ps = pd0_ps.tile([128, R], F32, tag="gps", name="gps")
                for rt in range(RT):
                    sl = bass.ts(rt, 512)
                    for p in range(8):
                        c = 2 * p
                        src = cat8(c)
                        cc = c if c < HC else c - HC
                        nc.tensor.matmul(ps[:, sl], gws[:, c:c + 2, :],
                                         src[:, cc:cc + 2, sl],
                                         start=(p == 0), stop=(p == 7),
                                         perf_mode=DR)
                nc.scalar.activation(out=sigb_sb[:, oc, :], in_=ps,
                                     func=AF.Sigmoid, scale=1.0 / WS,
                                     bias=gb_sb[:, oc:oc + 1])


        pBC.release()   # frees ksb/vsb/q8/x8/ctx8 (7 MB)

        # =========== Phase D2: layernorm apply + gelu (in place) ===========
        with ExitStack() as sd2:
            pd2 = sd2.enter_context(tc.tile_pool(name="pd2", bufs=3))
            for oc2 in range(H2C):
                t1 = pd2.tile([128, R], F32, tag="t1", name="t1")
                if oc2 % 2 == 0:
                    nc.gpsimd.tensor_sub(t1, h1_sb[:, oc2, :], mu2_f)
                else:
                    nc.vector.tensor_sub(t1, h1_sb[:, oc2, :], mu2_f)
                nc.vector.scalar_tensor_tensor(
                    out=t1, in0=t1, scalar=ilg_sb[:, oc2:oc2 + 1],
                    in1=rstd2_f, op0=OP.mult, op1=OP.mult)
                nc.scalar.activation(out=h1_sb[:, oc2, :], in_=t1,
                                     func=AF.Gelu,
                                     bias=ilb_sb[:, oc2:oc2 + 1])

        # ====== Phase D3+D4: integ, y = x + gate*integ, final layernorm ====
        with ExitStack() as sd3:
            pd3_big = sd3.enter_context(tc.tile_pool(name="pd3_big", bufs=1))
            yt_sb = pd3_big.tile([128, HC, R], F32R)           # 4 MB
            pd3_o = sd3.enter_context(tc.tile_pool(name="pd3_o", bufs=2))
            pd3_ps = sd3.enter_context(
                tc.tile_pool(name="pd3_ps", bufs=2, space="PSUM"))
            pd3_st = sd3.enter_context(
                tc.tile_pool(name="pd3_st", bufs=1, space="PSUM"))
            pd3_tp = sd3.enter_context(
                tc.tile_pool(name="pd3_tp", bufs=2, space="PSUM"))
            for rt in range(RT):
                sl = bass.ts(rt, 512)
                for oc in range(HC):
                    ps = pd3_ps.tile([128, 512], F32, tag="w2ps", name="w2ps")
                    for hc in range(H2C):
                        nc.tensor.matmul(ps, w2_all[:, oc, hc, :],
                                         h1_sb[:, hc, sl],
                                         start=(hc == 0), stop=(hc == H2C - 1))
                    ytmp = pd3_o.tile([128, 512], F32, tag="ytmp", name="ytmp")
                    nc.vector.scalar_tensor_tensor(
                        out=ytmp, in0=ps, scalar=b2_sb[:, oc:oc + 1],
                        in1=sigb_sb[:, oc, sl], op0=OP.add, op1=OP.mult)
                    nc.gpsimd.tensor_add(yt_sb[:, oc, sl], ytmp,
                                         xt_sb[:, oc, sl].bitcast(F32))
                # final layernorm for this row tile (gamma/beta folded in)
                muy = pd3_st.tile([128, 512], F32, tag="muy", name="muy")
                msy = pd3_st.tile([128, 512], F32, tag="msy", name="msy")
                for oc in range(HC):
                    sqy = pd3_o.tile([128, 512], F32R, tag="sqy", name="sqy")
                    nc.vector.tensor_mul(sqy, yt_sb[:, oc, sl].bitcast(F32),
                                         yt_sb[:, oc, sl].bitcast(F32))
                    nc.tensor.matmul(muy, ones1k, yt_sb[:, oc, sl],
                                     start=(oc == 0), stop=(oc == HC - 1))
                    nc.tensor.matmul(msy, ones1k, sqy,
                                     start=(oc == 0), stop=(oc == HC - 1))
                muy_f = pd3_o.tile([128, 512], F32, tag="muyf", name="muyf",
                                   bufs=1)
                nc.scalar.activation(out=muy_f, in_=muy, func=AF.Copy)
                var = pd3_o.tile([128, 512], F32, tag="vary", name="vary",
                                 bufs=1)
                nc.vector.tensor_mul(var, muy_f, muy_f)
                nc.vector.tensor_sub(var, msy, var)
                nc.scalar.activation(out=var, in_=var, func=AF.Ln,
                                     bias=eps_t, scale=1.0)
                rstdy = pd3_o.tile([128, 512], F32, tag="rsty", name="rstdy",
                                   bufs=1)
                nc.scalar.activation(out=rstdy, in_=var, func=AF.Exp,
                                     scale=-0.5)
                # y_norm = (y - mu)*rstd*l2g + l2b, all feature-major
                for oc in range(HC):
                    t = pd3_o.tile([128, 512], F32, tag="ynt", name="ynt")
                    nc.vector.tensor_sub(t, yt_sb[:, oc, sl].bitcast(F32),
                                         muy_f)
                    nc.vector.scalar_tensor_tensor(
                        out=t, in0=t, scalar=l2g_sb[:, oc:oc + 1],
                        in1=rstdy, op0=OP.mult, op1=OP.mult)
                    nc.scalar.activation(out=yt_sb[:, oc, sl], in_=t,
                                         func=AF.Identity,
                                         bias=l2b_sb[:, oc:oc + 1])
                for j in range(4):
                    rc = rt * 4 + j
                    c0 = rt * 512 + j * 128
                    tp = pd3_tp.tile([128, H], F32R, tag="tp", name="tp")
                    for oc in range(HC):
                        nc.tensor.transpose(
                            tp[:, oc * 128:(oc + 1) * 128],
                            yt_sb[:, oc, c0:c0 + 128], ident)
                    yr = pd3_o.tile([128, H], F32, tag="yr", name="yr")
                    nc.scalar.activation(out=yr, in_=tp.bitcast(F32),
                                         func=AF.Copy)
                    nc.sync.dma_start(out=d_out[rc * 128:(rc + 1) * 128, :],
                                      in_=yr)

    nc.compile()
    return nc


_NC_CACHE = []


def _get_nc():
    if not _NC_CACHE:
        _NC_CACHE.append(build_program())
    return _NC_CACHE[0]


def _fp8(a, scale=1.0):
    return np.asarray(np.asarray(a, np.float32) * scale).astype(
        ml_dtypes.float8_e4m3)


def _chunk(w_t, dtype, scale=1.0):
    # [IN, OUT] -> [OUT//128, 128, IN//128, 128] contiguous per-partition
    inn, out = w_t.shape
    r = (w_t * scale).reshape(inn // 128, 128, out // 128, 128)
    r = r.transpose(2, 1, 0, 3)
    return np.ascontiguousarray(r.astype(dtype))


def kernel(query_hidden, mem_keys, importance, recency, access_count,
           Wq, bq, in_w, in_b, out_w, out_b, gate_w, gate_b,
           int_w1, int_b1, int_ln_g, int_ln_b, int_w2, int_b2,
           ln1_g, ln1_b, ln2_g, ln2_b, sel_params, top_k):
    np32 = lambda a: np.asarray(a, dtype=np.float32)
    query_hidden = np32(query_hidden)
    mem_keys = np32(mem_keys)
    top_k = int(top_k)
    assert top_k == K, f"kernel compiled for top_k={K}, got {top_k}"

    # HTPS selection (host): softmax-weighted score, top-k set, gather.
    sp = np32(sel_params)
    w = np.exp(sp - sp.max())
    w = w / w.sum()
    acc = np32(access_count)
    sel = w[0] * np32(importance) + w[1] * np32(recency) + w[2] * (acc / acc.max())
    idx = np.argpartition(-sel, top_k - 1)[:top_k]
    mem = mem_keys[idx]                                  # [K, H]

    # memory layernorm on host (ln1 gamma/beta folded into wk/wv below)
    mu = mem.mean(1, keepdims=True)
    var = ((mem - mu) ** 2).mean(1, keepdims=True)
    mem_n = (mem - mu) / np.sqrt(var + EPS)
    mn8_t = np.ascontiguousarray(_fp8(mem_n.T))          # [H, K] fp8

    in_w = np32(in_w)
    in_b = np32(in_b)
    wq, wk, wv = in_w[:H], in_w[H:2 * H], in_w[2 * H:]
    bqi, bki, bvi = in_b[:H], in_b[H:2 * H], in_b[2 * H:]
    wc = wq @ np32(Wq)                                   # fused query proj
    bc = wq @ np32(bq) + bqi

    g1 = np32(ln1_g)
    b1v = np32(ln1_b)
    bki = bki + wk @ b1v
    bvi = bvi + wv @ b1v
    wk = wk * g1[None, :]
    wv = wv * g1[None, :]

    out_w = np32(out_w)
    out_b = np32(out_b)
    gate_w = np32(gate_w)
    int_w1 = np32(int_w1)
    gwx, gwa = gate_w[:, :H], gate_w[:, H:]
    w1x, w1a = int_w1[:, :H], int_w1[:, H:]
    gate_b_f = np32(gate_b) + gwa @ out_b
    int_b1_f = np32(int_b1) + w1a @ out_b

    T = lambda a: np.ascontiguousarray(np32(a).T)
    gw_t = np.concatenate([gwx.T, (gwa @ out_w).T], axis=0)   # [2H, H]
    w1_t = np.concatenate([w1x.T, (w1a @ out_w).T], axis=0)   # [2H, 2H]

    # wv8: [128, HC, H] = wv.T reshaped (in-chunk-major partitions)
    wv_t = (T(wv) * WS).reshape(HC, 128, H).transpose(1, 0, 2)
    common = {
        "mn8_t": mn8_t,
        "wc8": _chunk(T(wc), ml_dtypes.float8_e4m3, WS),
        "wk8": _chunk(T(wk), ml_dtypes.float8_e4m3, WS),
        "wv8": np.ascontiguousarray(wv_t.astype(ml_dtypes.float8_e4m3)),
        "gw8": _chunk(gw_t, ml_dtypes.float8_e4m3, WS),
        "w1b": _chunk(w1_t, ml_dtypes.float8_e4m3, WS),
        "w2b": _chunk(T(np32(int_w2)), ml_dtypes.bfloat16),
        "biases": np.concatenate([
            QS * bc, QS * bki, bvi, gate_b_f, np32(int_b2),
            np32(ln2_g), np32(ln2_b),
            int_b1_f, np32(int_ln_g), np32(int_ln_b)]).astype(np.float32),
    }
    X = query_hidden.reshape(B * S, H)
    in_maps = []
    for c in range(N_CORES):
        m = dict(common)
        xc_t = np.ascontiguousarray(X[c * R:(c + 1) * R].T)
        m["x_t"] = xc_t
        m["x8_t"] = np.ascontiguousarray(_fp8(xc_t))
        in_maps.append(m)

    nc = _get_nc()
    res = run_bass_kernel_spmd(nc, in_maps, core_ids=list(range(N_CORES)))
    out = np.empty((B * S, H), dtype=np.float32)
    for c in range(N_CORES):
        out[c * R:(c + 1) * R] = res.results[c]["out"]
    return out.reshape(B, S, H)
